# revision 4
# baseline (speedup 1.0000x reference)
"""Trainium2 Bass kernel for nn_EstimatorCRF: BiGRU x2 -> cross/self attention -> emit -> CRF.

v2: sequence-parallel GRU. Each direction's scan is split into NSEG=4 segments
with a W-step warmup (GRU state decays fast, warmup is exact for seg 0 via
zero-forced gates and near-exact elsewhere since warmup gx is the true gx of
the preceding W steps, computed on host). 8 cores = 4 directions x 2 core
halves; each core runs CH=NSEG/2 independent chains interleaved in one
hardware loop so cross-engine latency of one chain hides under the other.

  - AG #1: feat-major [feat, t] blocks (per-core: CH segments side by side).
  - AG #2: natural [t, feat] bf16 blocks.
  - attention/emit q-sharded (QB = T/8 rows per core), K cached in SBUF.
  - AG #3: emit rows; CRF + gold replicated; scalar output.
"""

import sys
for _p in ("/opt/trn_rl_repo",):
    if _p not in sys.path:
        sys.path.insert(0, _p)


import numpy as np
import ml_dtypes

import concourse.bass as bass
import concourse.mybir as mybir
import concourse.tile as tile
from concourse import bacc
from concourse.bass import ds, ts
from concourse.masks import make_identity

FP32 = mybir.dt.float32
BF16 = mybir.dt.bfloat16
AF = mybir.ActivationFunctionType
ALU = mybir.AluOpType
AX = mybir.AxisListType

NEG_BIG = -1.0e30


class Cfg:
    def __init__(self, T=2048, E=512, H=512, U=32, n_cores=8, NSEG=4, W=32,
                 debug_outs=False, skip_bhhn=False, stop_after=None):
        self.T, self.E, self.H, self.U, self.NC = T, E, H, U, n_cores
        self.NSEG = NSEG
        self.CH = NSEG // 2           # chains per core
        self.SW = T // NSEG           # real steps per chain
        self.W = W                    # warmup steps per chain
        self.Tsc = self.SW + W        # total scan steps per chain
        self.debug_outs = debug_outs
        self.skip_bhhn = skip_bhhn
        self.stop_after = stop_after
        assert E == 512 and H == 512, "layout hardcoded for E=H=512"
        self.HC = H // 128            # h chunks (4)
        self.GC = 3 * self.HC         # gate chunks (12)
        self.G3 = 3 * H               # 1536
        self.QB = T // n_cores        # q rows per core
        assert self.QB % 128 == 0
        self.QTN = self.QB // 128
        self.FW = T // 128
        assert self.Tsc % U == 0
        assert self.SW % 128 == 0


# ----------------------------------------------------------------------------
# Host-side input preparation
# ----------------------------------------------------------------------------

def prep_in_maps(inputs, cfg: Cfg):
    T, H, E = cfg.T, cfg.H, cfg.E
    CH, SW, W, GC = cfg.CH, cfg.SW, cfg.W, cfg.GC
    f32 = np.float32
    bf16 = ml_dtypes.bfloat16
    d = {k: np.asarray(v) for k, v in inputs.items()}

    dirs = []  # (x [T,E] scan-ordered, rev, Wih, Whh, bih, bhh)
    for enc, xname in (("src", "source"), ("tgt", "target")):
        x = d[xname][0]
        for dr, rev in (("f", False), ("b", True)):
            xs = x[::-1] if rev else x
            dirs.append((xs, rev, d[f"{enc}_Wih_{dr}"], d[f"{enc}_Whh_{dr}"],
                         d[f"{enc}_bih_{dr}"], d[f"{enc}_bhh_{dr}"]))

    labels = np.asarray(d["labels"]).astype(np.int64)
    lab = labels.astype(f32)
    FW = cfg.FW

    def tplane(v):
        return np.ascontiguousarray(v.reshape(128, FW).astype(f32))

    laba = np.zeros(T, f32); laba[:T - 1] = lab[1:]
    labb = np.zeros(T, f32); labb[:T - 1] = lab[:T - 1]

    t_trans = d["t_trans"].astype(f32)
    ttrans_b = np.tile(t_trans.reshape(1, 4), (128, 1)).astype(f32)
    tstart_b = np.tile(d["t_start"].reshape(1, 2), (128, 1)).astype(f32)
    tend_b = np.tile(d["t_end"].reshape(1, 2), (128, 1)).astype(f32)
    wemitT = np.ascontiguousarray(d["W_emit"].astype(f32).T).astype(bf16)
    bemit = np.tile(d["b_emit"].reshape(1, 2), (128, 1)).astype(f32)

    in_maps = []
    NSEG = cfg.NSEG
    for c in range(cfg.NC):
        xs, rev, Wih, Whh, bih, bhh = dirs[c % 4]
        wihT = np.ascontiguousarray(Wih.astype(f32).T)         # [E, 3H]
        whhT = np.ascontiguousarray(Whh.astype(f32).T).astype(bf16)
        gxb = bih.astype(f32).copy()
        gxb[:2 * H] += bhh[:2 * H].astype(f32)                 # fold bhh_{r,z}
        gxbias = np.ascontiguousarray(gxb.reshape(GC, 128).T)  # [128, GC]
        bhhn = np.ascontiguousarray(bhh[2 * H:].astype(f32).reshape(cfg.HC, 128).T)

        # per-chain real inputs [E, CH*SW] and warmup gx [128, CH*GC*W]
        xT = np.zeros((E, CH * SW), f32)  # converted to bf16 below
        gxw = np.zeros((128, CH * GC * W), f32)
        for j in range(CH):
            sg = CH * (c // 4) + j            # true segment this chain owns
            s = (NSEG - 1 - sg) if rev else sg  # scan-order segment
            xT[:, j * SW:(j + 1) * SW] = xs[s * SW:(s + 1) * SW].T
            if s == 0:
                g = np.zeros((W, 3 * H), f32)
                g[:, H:2 * H] = 30.0       # z ~= 1 and gxn = 0: h stays 0
            else:
                g = xs[s * SW - W:s * SW].astype(f32) @ wihT + gxb  # [W, 3H]
            # [W, 3H] -> [128, GC, W]  (gate g3 = cchunk*128 + p)
            gw = np.ascontiguousarray(g.T.reshape(GC, 128, W).transpose(1, 0, 2))
            gxw[:, j * GC * W:(j + 1) * GC * W] = gw.reshape(128, GC * W)

        qoff = c * cfg.QB
        dm = np.zeros((cfg.QB, T), f32)
        for i in range(cfg.QB):
            dm[i, qoff + i] = NEG_BIG

        in_maps.append(dict(
            xT=xT.astype(bf16), wihT=wihT.astype(bf16), whhT=whhT,
            gxbias=gxbias, bhhn=bhhn,
            gxwarm=gxw.astype(bf16),
            diagmask=dm, wemitT=wemitT, bemit=bemit,
            ttrans_b=ttrans_b, tstart_b=tstart_b, tend_b=tend_b,
            lab16=tplane(lab), laba16=tplane(laba), labb16=tplane(labb),
            labends=np.tile(np.array([[lab[0], lab[T - 1]]], f32), (128, 1)),
        ))
    return in_maps


# ----------------------------------------------------------------------------
# Kernel builder
# ----------------------------------------------------------------------------

def build(nc: bacc.Bacc, tc: tile.TileContext, cfg: Cfg):
    T, E, H, U = cfg.T, cfg.E, cfg.H, cfg.U
    HC, GC, G3, QB, QTN, FW = cfg.HC, cfg.GC, cfg.G3, cfg.QB, cfg.QTN, cfg.FW
    CH, SW, W, Tsc, NSEG = cfg.CH, cfg.SW, cfg.W, cfg.Tsc, cfg.NSEG
    Tsc1 = Tsc + 1
    NK = E // 128
    NC = cfg.NC
    _ORDER = ["none", "gx", "scan", "ag", "attnq", "attnk", "attn",
              "emit", "ag3", None]
    lim = _ORDER.index(cfg.stop_after)

    def din(name, shape, dt=FP32):
        return nc.dram_tensor(name, list(shape), dt, kind="ExternalInput")

    xT_d = din("xT", (E, CH * SW), BF16)
    wihT_d = din("wihT", (E, G3), BF16)
    whhT_d = din("whhT", (H, G3), BF16)
    gxbias_d = din("gxbias", (128, GC))
    bhhn_d = din("bhhn", (128, HC))
    gxwarm_d = din("gxwarm", (128, CH * GC * W), BF16)
    diag_d = din("diagmask", (QB, T))
    wemitT_d = din("wemitT", (6 * H, 2), BF16)
    bemit_d = din("bemit", (128, 2))
    ttrans_d = din("ttrans_b", (128, 4))
    tstart_d = din("tstart_b", (128, 2))
    tend_d = din("tend_b", (128, 2))
    lab_d = din("lab16", (128, FW))
    laba_d = din("laba16", (128, FW))
    labb_d = din("labb16", (128, FW))
    labends_d = din("labends", (128, 2))

    out_d = nc.dram_tensor("out_scalar", [1, 1], FP32, kind="ExternalOutput")
    if cfg.debug_outs:
        dbg_emit_d = nc.dram_tensor("dbg_emit", [T, 2], FP32, kind="ExternalOutput")
        dbg_ys_d = nc.dram_tensor("dbg_ys", [CH * 2 * HC * 128, Tsc1], FP32,
                                  kind="ExternalOutput")

    pid = nc.partition_id()
    sel = pid % 2              # 1 on backward-direction cores

    # static map: true segment sg of direction dd -> (ag block, col base)
    # (host assigns bwd chains reversed segments, so this is dir-independent)
    def seg_src(dd, sg):
        return dd + 4 * (sg // CH), (sg % CH) * SW

    # ---- persistent small SBUF ----
    pers = tc.alloc_tile_pool(name="pers", bufs=1)
    whh_sb = pers.tile([128, HC * G3], BF16, tag="whh")
    gxbias_sb = pers.tile([128, GC], FP32, tag="gxbias")
    bhhn_sb = pers.tile([128, HC], FP32, tag="bhhn")
    ident = pers.tile([128, 128], FP32, tag="ident")
    make_identity(nc, ident[:])
    ident_bf = pers.tile([128, 128], BF16, tag="identbf")
    nc.vector.tensor_copy(ident_bf[:], ident[:])
    ttrans_sb = pers.tile([128, 4], FP32, tag="ttr")
    tstart_sb = pers.tile([128, 2], FP32, tag="tst")
    tend_sb = pers.tile([128, 2], FP32, tag="ten")
    lab_sb = pers.tile([128, FW], FP32, tag="lab")
    laba_sb = pers.tile([128, FW], FP32, tag="laba")
    labb_sb = pers.tile([128, FW], FP32, tag="labb")
    bemit_sb = pers.tile([128, 2], FP32, tag="bemit")
    wemit_sb = pers.tile([128, (6 * H // 128) * 2], BF16, tag="wemit")
    labends_sb = pers.tile([128, 2], FP32, tag="labends")

    nc.sync.dma_start(whh_sb[:], whhT_d.ap().rearrange("(k p) g -> p k g", p=128))
    nc.sync.dma_start(gxbias_sb[:], gxbias_d[:, :])
    nc.sync.dma_start(bhhn_sb[:], bhhn_d[:, :])
    nc.sync.dma_start(ttrans_sb[:], ttrans_d[:, :])
    nc.sync.dma_start(tstart_sb[:], tstart_d[:, :])
    nc.sync.dma_start(tend_sb[:], tend_d[:, :])
    nc.sync.dma_start(lab_sb[:], lab_d[:, :])
    nc.sync.dma_start(laba_sb[:], laba_d[:, :])
    nc.sync.dma_start(labb_sb[:], labb_d[:, :])
    nc.sync.dma_start(bemit_sb[:], bemit_d[:, :])
    nc.sync.dma_start(labends_sb[:], labends_d[:, :])
    nc.sync.dma_start(wemit_sb[:], wemitT_d.ap().rearrange("(k p) c -> p k c", p=128))

    # ---- DRAM pools for collectives ----
    dram = tc.alloc_tile_pool(name="dram", bufs=1, space="DRAM")
    b1_in = dram.tile([HC * 128, CH * SW], BF16, tag="b1i")
    ag1 = dram.tile([NC * HC * 128, CH * SW], BF16, tag="ag1", addr_space="Shared")
    b3_in = dram.tile([QB, 2], FP32, tag="b3i")
    ag3 = dram.tile([NC * QB, 2], FP32, tag="ag3", addr_space="Shared")

    ys_pool = tc.alloc_tile_pool(name="ysp", bufs=1)
    ys = ys_pool.tile([128, CH * 2 * HC * Tsc1], FP32, tag="ys")
    ysv = ys[:].rearrange("p (a t) -> p a t", a=CH * 2 * HC)

    # ============================ phase 1 + 2 =================================
    with tc.tile_pool(name="gxp", bufs=1) as gxp:
        gx_sb = gxp.tile([128, CH * GC * Tsc], BF16, tag="gx")
        gxv = gx_sb[:].rearrange("p (c t) -> p c t", c=CH * GC)

        # warmup gx columns from host
        nc.sync.dma_start(gxv[:, :, 0:W],
                          gxwarm_d.ap().rearrange("p (c w) -> p c w", c=CH * GC))

        if lim >= 1:
            with tc.tile_pool(name="ph1", bufs=1) as ph1, \
                 tc.tile_pool(name="ph1ps", bufs=2, space="PSUM") as ph1ps:
                xT_sb = ph1.tile([128, NK * CH * SW], BF16, tag="xT")
                wih_sb = ph1.tile([128, NK * G3], BF16, tag="wih")
                nc.sync.dma_start(
                    xT_sb[:], xT_d.ap().rearrange("(k p) t -> p k t", p=128))
                nc.sync.dma_start(
                    wih_sb[:], wihT_d.ap().rearrange("(k p) g -> p k g", p=128))

                CHK = min(SW, 512)
                for ch in range(CH):
                    for c in range(GC):
                        for n in range(SW // CHK):
                            ps = ph1ps.tile([128, CHK], FP32, tag="gxps")
                            for k in range(NK):
                                nc.tensor.matmul(
                                    ps[:, :],
                                    wih_sb[:, k * G3 + c * 128:
                                           k * G3 + (c + 1) * 128],
                                    xT_sb[:, k * CH * SW + ch * SW + n * CHK:
                                          k * CH * SW + ch * SW + (n + 1) * CHK],
                                    start=(k == 0), stop=(k == NK - 1))
                            nc.vector.tensor_scalar_add(
                                gxv[:, ch * GC + c,
                                    W + n * CHK:W + (n + 1) * CHK],
                                ps[:, :], gxbias_sb[:, c:c + 1])

        # ---- GRU scan: CH interleaved chains ----
        if lim >= 2:
            for ch in range(CH):
                nc.vector.memset(ysv[:, ch * 2 * HC:(ch + 1) * 2 * HC, 0:1], 0.0)
            skip_bhhn = cfg.skip_bhhn
            with tc.tile_pool(name="scan", bufs=3) as scp, \
                 tc.tile_pool(name="scanps0", bufs=2, space="PSUM") as pspl0, \
                 tc.tile_pool(name="scanps1", bufs=2, space="PSUM") as pspl1, \
                 tc.tile_pool(name="scanps2", bufs=2, space="PSUM") as pspl2, \
                 tc.tile_pool(name="scanps3", bufs=2, space="PSUM") as pspl3:
                pspl = [pspl0, pspl1, pspl2, pspl3][:CH]
                with tc.For_i(0, Tsc, U, staggered_reset=True,
                              hint_engines=(mybir.EngineType.PE,
                                            mybir.EngineType.DVE)) as iv:
                    hbf_prev = [None] * CH
                    for u in range(U):
                        j = iv + u
                        A0 = [ch * 2 * HC for ch in range(CH)]
                        G0 = [ch * GC for ch in range(CH)]
                        hbf = []
                        for ch in range(CH):
                            if u > 0:
                                hbf.append(hbf_prev[ch])
                            else:
                                h = scp.tile([128, HC], BF16, tag=f"hbf{ch}")
                                nc.vector.tensor_copy(
                                    h[:], ysv[:, A0[ch]:A0[ch] + HC, ds(j, 1)])
                                hbf.append(h)
                        pAB = [pspl[ch].tile([128, 12], FP32, tag=f"pAB{ch}",
                                             name=f"pAB{ch}")
                               for ch in range(CH)]
                        pA = [t[:, 0:8] for t in pAB]
                        pB = [t[:, 8:12] for t in pAB]
                        # r-gate matmuls, both chains
                        for ch in range(CH):
                            for c in range(0, 4):
                                for k in range(HC):
                                    nc.tensor.matmul(
                                        pA[ch][:, c:c + 1],
                                        whh_sb[:, k * G3 + c * 128:
                                               k * G3 + (c + 1) * 128],
                                        hbf[ch][:, k:k + 1], start=(k == 0),
                                        stop=False)
                                nc.tensor.matmul(
                                    pA[ch][:, c:c + 1], ident_bf[:],
                                    gxv[:, G0[ch] + c, ds(j, 1)],
                                    start=False, stop=True)
                        sr = [scp.tile([128, 4], FP32, tag=f"sr{ch}", name=f"sr{ch}")
                              for ch in range(CH)]
                        for ch in range(CH):
                            nc.scalar.activation(sr[ch][:], pA[ch][:, 0:4],
                                                 AF.Sigmoid)
                        # n- and z-gate matmuls, both chains
                        for ch in range(CH):
                            for c in range(8, 12):
                                for k in range(HC):
                                    nc.tensor.matmul(
                                        pB[ch][:, c - 8:c - 7],
                                        whh_sb[:, k * G3 + c * 128:
                                               k * G3 + (c + 1) * 128],
                                        hbf[ch][:, k:k + 1], start=(k == 0),
                                        stop=(k == HC - 1))
                            for c in range(4, 8):
                                for k in range(HC):
                                    nc.tensor.matmul(
                                        pA[ch][:, c:c + 1],
                                        whh_sb[:, k * G3 + c * 128:
                                               k * G3 + (c + 1) * 128],
                                        hbf[ch][:, k:k + 1], start=(k == 0),
                                        stop=False)
                                nc.tensor.matmul(
                                    pA[ch][:, c:c + 1], ident_bf[:],
                                    gxv[:, G0[ch] + c, ds(j, 1)],
                                    start=False, stop=True)
                        sz = [scp.tile([128, 4], FP32, tag=f"sz{ch}", name=f"sz{ch}")
                              for ch in range(CH)]
                        for ch in range(CH):
                            nc.scalar.activation(sz[ch][:], pA[ch][:, 4:8],
                                                 AF.Sigmoid)
                        tn2 = [scp.tile([128, HC], FP32, tag=f"tn2{ch}", name=f"tn2{ch}")
                               for ch in range(CH)]
                        tn3 = [scp.tile([128, HC], FP32, tag=f"tn3{ch}", name=f"tn3{ch}")
                               for ch in range(CH)]
                        for ch in range(CH):
                            if skip_bhhn:
                                nsrc = pB[ch][:, :]
                            else:
                                tn1 = scp.tile([128, HC], FP32, tag=f"tn1{ch}")
                                nc.vector.tensor_tensor(tn1[:], pB[ch][:, :],
                                                        bhhn_sb[:], ALU.add)
                                nsrc = tn1[:]
                            nc.vector.tensor_tensor(tn2[ch][:], nsrc, sr[ch][:],
                                                    ALU.mult)
                            nc.vector.tensor_tensor(
                                tn3[ch][:], tn2[ch][:],
                                gxv[:, G0[ch] + 8:G0[ch] + 12, ds(j, 1)], ALU.add)
                        nn = [scp.tile([128, HC], FP32, tag=f"nn{ch}", name=f"nn{ch}")
                              for ch in range(CH)]
                        for ch in range(CH):
                            nc.scalar.activation(nn[ch][:], tn3[ch][:], AF.Tanh)
                        zm1 = [scp.tile([128, 4], FP32, tag=f"zm1{ch}", name=f"zm1{ch}")
                               for ch in range(CH)]
                        t1 = [scp.tile([128, HC], FP32, tag=f"t1{ch}", name=f"t1{ch}")
                              for ch in range(CH)]
                        for ch in range(CH):
                            nc.vector.tensor_scalar_sub(zm1[ch][:], sz[ch][:], 1.0)
                            nc.vector.tensor_tensor(
                                t1[ch][:], sz[ch][:],
                                ysv[:, A0[ch]:A0[ch] + HC, ds(j, 1)], ALU.mult)
                        for ch in range(CH):
                            t2 = scp.tile([128, HC], FP32, tag=f"t2{ch}")
                            nc.vector.tensor_tensor(t2[:], nn[ch][:], zm1[ch][:],
                                                    ALU.mult)
                            hbf2 = scp.tile([128, HC], BF16, tag=f"hbf2{ch}")
                            nc.vector.tensor_tensor(hbf2[:], t1[ch][:], t2[:],
                                                    ALU.subtract)
                            nc.vector.tensor_tensor(
                                ysv[:, A0[ch]:A0[ch] + HC, ds(j + 1, 1)],
                                t1[ch][:], t2[:], ALU.subtract)
                            nc.vector.tensor_copy(
                                ysv[:, A0[ch] + HC:A0[ch] + 2 * HC,
                                    ds((Tsc - u) - iv, 1)],
                                ysv[:, A0[ch]:A0[ch] + HC, ds(j + 1, 1)])
                            hbf_prev[ch] = hbf2

    if cfg.debug_outs:
        nc.sync.dma_start(dbg_ys_d.ap().rearrange("(a p) t -> p a t", p=128),
                          ysv[:, :, :])

    # =================== phase 3: natural transposes + AGs ===================
    if lim >= 3:
        natp = tc.alloc_tile_pool(name="natp", bufs=1)
        conv = natp.tile([128, CH * HC * SW], BF16, tag="conv")
        for ch in range(CH):
            a0 = ch * 2 * HC
            nc.vector.tensor_copy(
                conv[:].rearrange("p (c k t) -> p c k t", c=CH, k=HC)[:, ch, :, :],
                ysv[:, ds(a0 + sel * HC, HC), ds(1 + W - W * sel, SW)])
        for ch in range(CH):
            nc.gpsimd.dma_start(
                b1_in[:, ch * SW:(ch + 1) * SW]
                .rearrange("(k p) t -> p k t", p=128),
                conv[:].rearrange("p (c k t) -> p c k t", c=CH, k=HC)[:, ch, :, :])
        nc.gpsimd.collective_compute(
            "AllGather", ALU.bypass, ins=[b1_in.opt()], outs=[ag1.opt()],
            replica_groups=[list(range(NC))])

        natp.release()
    ys_pool.release()

    # ===================== phase 4: attention (q-sharded) =====================
    att = tc.alloc_tile_pool(name="att", bufs=1)
    qt_sb = att.tile([128, 8 * QB], BF16, tag="qt")
    qt_cand = att.tile([128, 2 * 8 * QB], BF16, tag="qtc")
    diag_sb = att.tile([128, QTN * T], FP32, tag="diag")
    featsT = att.tile([128, 24 * QB], BF16, tag="featsT")
    pt_sb = att.tile([128, (T // 128) * QB], BF16, tag="ptq")
    natfull = att.tile([128, (T // 128) * 1024], BF16, tag="natfull")
    kfull = att.tile([128, 8 * T], BF16, tag="kfull")
    emit_sb = att.tile([128, QTN * 2], FP32, tag="emit")

    if lim >= 4:
        # queries: rows [pid*QB, pid*QB+QB) of tgt features, runtime-stitched
        ratio = SW // QB                       # power of 2
        sh = ratio.bit_length() - 1
        sgq = pid >> sh                        # true segment of this q range
        oq = (pid & (ratio - 1)) * QB          # offset within segment
        cb = nc.s_assert_within((sgq & (CH - 1)) * SW + oq, 0, CH * SW - QB,
                                skip_runtime_assert=True)
        # runtime rows on a Shared tile crash the DMA engine: load both core
        # halves' candidate blocks (static rows, runtime col) and select after
        for hh in range(2):
            nc.gpsimd.dma_start(
                qt_cand[:, hh * 8 * QB:(hh + 1) * 8 * QB]
                .rearrange("p (k q) -> p k q", k=8),
                ag1[(2 + 4 * hh) * 512:(4 + 4 * hh) * 512, ds(cb, QB)]
                .rearrange("(k p) t -> p k t", p=128))
        sgh = sgq >> (CH.bit_length() - 1)     # which core half owns it
        qoff = nc.s_assert_within(sgh * (8 * QB), 0, 8 * QB,
                                  skip_runtime_assert=True)
        nc.vector.tensor_copy(qt_sb[:], qt_cand[:, ds(qoff, 8 * QB)])
        nc.vector.tensor_copy(featsT[:, 0:8 * QB], qt_sb[:])
        nc.vector.tensor_scalar_mul(qt_sb[:], qt_sb[:], float(np.sqrt(2.0 * H)))
        nc.sync.dma_start(diag_sb[:].rearrange("p (q t) -> p q t", q=QTN),
                          diag_d.ap().rearrange("(q p) t -> p q t", p=128))

    with tc.tile_pool(name="psS", bufs=1, space="PSUM") as psS, \
         tc.tile_pool(name="psT", bufs=2, space="PSUM") as psT, \
         tc.tile_pool(name="psC", bufs=2, space="PSUM") as psC, \
         tc.tile_pool(name="Pp", bufs=2) as Pp, \
         tc.tile_pool(name="attsm", bufs=4) as attsm:
        if lim >= 5:
            ST = T // 128
            for at in range(2):
                if at == 1 and lim < 6:
                    continue
                kv0 = 0 if at == 0 else 2
                # K cache: [feat-chunk kt][true t] for this attention's 2 dirs
                for dpart in range(2):
                    for sg in range(NSEG):
                        blk, cb = seg_src(kv0 + dpart, sg)
                        nc.sync.dma_start(
                            kfull[:, :].rearrange("p (a t) -> p a t", a=8)
                            [:, dpart * NK:(dpart + 1) * NK, sg * SW:(sg + 1) * SW],
                            ag1[blk * 512:(blk + 1) * 512, cb:cb + SW]
                            .rearrange("(k p) t -> p k t", p=128))
                # V natural tiles: transpose the K cache on-core
                for st in range(ST):
                    for m in range(8):
                        kt = (m // NK) * NK + (m % NK)
                        ntp = psT.tile([128, 128], BF16, tag="ptp")
                        nc.tensor.transpose(
                            ntp[:], kfull[:, kt * T + st * 128:
                                          kt * T + (st + 1) * 128], ident_bf[:])
                        nc.vector.tensor_copy(
                            natfull[:, st * 1024 + m * 128:
                                    st * 1024 + (m + 1) * 128], ntp[:])
                if lim < 6:
                    continue
                for qi in range(QTN):
                    pS = [psS.tile([128, T // 2], FP32, tag=f"pS{sh}",
                                   name=f"pS{sh}") for sh in range(2)]
                    for kt in range(8):
                        for nch in range(T // 512):
                            sh = nch // (T // 1024)
                            off = (nch % (T // 1024)) * 512
                            nc.tensor.matmul(
                                pS[sh][:, off:off + 512],
                                qt_sb[:, kt * QB + qi * 128:
                                      kt * QB + (qi + 1) * 128],
                                kfull[:, kt * T + nch * 512:
                                      kt * T + (nch + 1) * 512],
                                start=(kt == 0), stop=(kt == 7))
                    if at == 1:
                        for sh in range(2):
                            nc.vector.tensor_tensor(
                                pS[sh][:, :], pS[sh][:, :],
                                diag_sb[:, qi * T + sh * (T // 2):
                                        qi * T + (sh + 1) * (T // 2)],
                                ALU.add)
                    mx = [attsm.tile([128, 1], FP32, tag=f"mx{sh}",
                                     name=f"mx{sh}") for sh in range(2)]
                    for sh in range(2):
                        nc.vector.reduce_max(mx[sh][:], pS[sh][:, :], AX.X)
                    negm = attsm.tile([128, 1], FP32, tag="negm")
                    nc.vector.tensor_tensor(negm[:], mx[0][:], mx[1][:], ALU.max)
                    nc.vector.tensor_scalar_mul(negm[:], negm[:], -1.0)
                    Pt = Pp.tile([128, T], FP32, tag="P")
                    Ptb = Pp.tile([128, T], BF16, tag="Pb")
                    sm = [attsm.tile([128, 1], FP32, tag=f"sm{sh}",
                                     name=f"sm{sh}") for sh in range(2)]
                    for sh in range(2):
                        nc.scalar.activation(
                            Pt[:, sh * (T // 2):(sh + 1) * (T // 2)],
                            pS[sh][:, :], AF.Exp, bias=negm[:],
                            accum_out=sm[sh][:])
                    smc = attsm.tile([128, 1], FP32, tag="smc")
                    nc.vector.tensor_tensor(smc[:], sm[0][:], sm[1][:], ALU.add)
                    rinv = attsm.tile([128, 1], FP32, tag="rinv")
                    nc.vector.reciprocal(rinv[:], smc[:])
                    nc.vector.tensor_scalar_mul(Ptb[:, :], Pt[:, :], rinv[:])
                    for st in range(ST):
                        ptp = psT.tile([128, 128], BF16, tag="ptp")
                        nc.tensor.transpose(ptp[:], Ptb[:, st * 128:(st + 1) * 128],
                                            ident_bf[:])
                        nc.vector.tensor_copy(
                            pt_sb[:, st * QB + qi * 128:
                                  st * QB + (qi + 1) * 128],
                            ptp[:])
                for m in range(8):
                    pc = psC.tile([128, QB], FP32, tag="pc")
                    for st in range(ST):
                        nc.tensor.matmul(
                            pc[:],
                            natfull[:, st * 1024 + m * 128:
                                    st * 1024 + (m + 1) * 128],
                            pt_sb[:, st * QB:(st + 1) * QB],
                            start=(st == 0), stop=(st == ST - 1))
                    nc.vector.tensor_copy(
                        featsT[:, (8 + at * 8 + m) * QB:(9 + at * 8 + m) * QB],
                        pc[:])

        if lim >= 7:
            for qi in range(QTN):
                pe = psC.tile([128, 2], FP32, tag="pc")
                for kt in range(24):
                    nc.tensor.matmul(
                        pe[:, :],
                        featsT[:, kt * QB + qi * 128: kt * QB + (qi + 1) * 128],
                        wemit_sb[:, kt * 2:(kt + 1) * 2],
                        start=(kt == 0), stop=(kt == 23))
                nc.vector.tensor_tensor(emit_sb[:, qi * 2:(qi + 1) * 2], pe[:, :],
                                        bemit_sb[:], ALU.add)

    if lim >= 8:
        nc.gpsimd.dma_start(b3_in[:].rearrange("(q p) c -> p q c", p=128),
                            emit_sb[:].rearrange("p (q c) -> p q c", q=QTN))
        nc.gpsimd.collective_compute(
            "AllGather", ALU.bypass, ins=[b3_in.opt()], outs=[ag3.opt()],
            replica_groups=[list(range(NC))])
    if cfg.debug_outs:
        nc.sync.dma_start(dbg_emit_d[:, :], ag3[0:T, :])

    # ========================= phase 5: CRF + gold ===========================
    if lim >= 9:
        crf = tc.alloc_tile_pool(name="crf", bufs=1)
        crfps = tc.alloc_tile_pool(name="crfps", bufs=2, space="PSUM")
        ep = [crf.tile([128, FW], FP32, tag=f"ep{i}", name=f"ep{i}")
              for i in range(2)]
        for i in range(2):
            nc.sync.dma_start(
                ep[i][:],
                ag3[0:T, :].rearrange("(p f) c -> p f c", p=128)[:, :, i:i + 1])

        pl = [[crf.tile([128, FW], FP32, tag=f"pl{i}{j}", name=f"pl{i}{j}")
               for j in range(2)] for i in range(2)]
        for i in range(2):
            for j in range(2):
                nc.vector.tensor_scalar_add(pl[i][j][:], ep[i][:],
                                            ttrans_sb[:, 2 * i + j: 2 * i + j + 1])
        for i in range(2):
            for j in range(2):
                nc.vector.tensor_tensor(pl[i][j][0:1, 0:1], ep[i][0:1, 0:1],
                                        tstart_sb[0:1, i:i + 1], ALU.add)

        cur = pl
        Wf = FW
        lvl = 0
        while Wf > 1:
            Wf //= 2
            nxt = [[crf.tile([128, Wf], FP32, tag=f"lv{lvl}_{i}{j}",
                             name=f"lv{lvl}_{i}{j}") for j in range(2)]
                   for i in range(2)]
            Aap = [[cur[k][j][:, 0:2 * Wf]
                    .rearrange("p (m two) -> p m two", two=2)[:, :, 0:1]
                    for j in range(2)] for k in range(2)]
            Bap = [[cur[i][k][:, 0:2 * Wf]
                    .rearrange("p (m two) -> p m two", two=2)[:, :, 1:2]
                    for k in range(2)] for i in range(2)]
            for i in range(2):
                for j in range(2):
                    X = crf.tile([128, Wf], FP32, tag=f"Xf{lvl}{i}{j}")
                    Y = crf.tile([128, Wf], FP32, tag=f"Yf{lvl}{i}{j}")
                    nc.vector.tensor_tensor(X[:], Bap[i][0], Aap[0][j], ALU.add)
                    nc.vector.tensor_tensor(Y[:], Bap[i][1], Aap[1][j], ALU.add)
                    M = crf.tile([128, Wf], FP32, tag=f"Mf{lvl}{i}{j}")
                    nc.vector.tensor_tensor(M[:], X[:], Y[:], ALU.max)
                    mn = crf.tile([128, Wf], FP32, tag=f"mnf{lvl}{i}{j}")
                    nc.vector.tensor_tensor(mn[:], X[:], Y[:], ALU.min)
                    dm = crf.tile([128, Wf], FP32, tag=f"dmf{lvl}{i}{j}")
                    nc.vector.tensor_tensor(dm[:], mn[:], M[:], ALU.subtract)
                    spe = crf.tile([128, Wf], FP32, tag=f"spef{lvl}{i}{j}")
                    nc.scalar.activation(spe[:], dm[:], AF.Exp)
                    sp = crf.tile([128, Wf], FP32, tag=f"spf{lvl}{i}{j}")
                    nc.scalar.activation(sp[:], spe[:], AF.Ln, bias=1.0)
                    nc.vector.tensor_tensor(nxt[i][j][:], M[:], sp[:], ALU.add)
            cur = nxt
            lvl += 1

        r128 = [[None, None], [None, None]]
        for i in range(2):
            for j in range(2):
                tps = crfps.tile([128, 128], FP32, tag="tps", name=f"tps{i}{j}")
                nc.tensor.transpose(tps[0:1, :], cur[i][j][:, 0:1], ident[:])
                rr = crf.tile([1, 128], FP32, tag=f"r128_{i}{j}",
                              name=f"r128_{i}{j}")
                nc.vector.tensor_copy(rr[:], tps[0:1, :])
                r128[i][j] = rr

        curp = r128
        curW = 128
        lvl = 0
        while curW > 1:
            curW //= 2
            nxtp = [[crf.tile([1, curW], FP32, tag=f"p4_{lvl}{i}{j}",
                              name=f"p4_{lvl}{i}{j}") for j in range(2)]
                    for i in range(2)]
            Aap = [[curp[k][j][0:1, 0:2 * curW]
                    .rearrange("p (m two) -> p m two", two=2)[:, :, 0:1]
                    for j in range(2)] for k in range(2)]
            Bap = [[curp[i][k][0:1, 0:2 * curW]
                    .rearrange("p (m two) -> p m two", two=2)[:, :, 1:2]
                    for k in range(2)] for i in range(2)]
            for i in range(2):
                for j in range(2):
                    X = crf.tile([1, curW], FP32, tag=f"X4{lvl}{i}{j}",
                                 name=f"X4{lvl}{i}{j}")
                    Y = crf.tile([1, curW], FP32, tag=f"Y4{lvl}{i}{j}",
                                 name=f"Y4{lvl}{i}{j}")
                    nc.vector.tensor_tensor(X[:], Bap[i][0], Aap[0][j], ALU.add)
                    nc.vector.tensor_tensor(Y[:], Bap[i][1], Aap[1][j], ALU.add)
                    M = crf.tile([1, curW], FP32, tag=f"M4{lvl}{i}{j}",
                                 name=f"M4{lvl}{i}{j}")
                    nc.vector.tensor_tensor(M[:], X[:], Y[:], ALU.max)
                    mn = crf.tile([1, curW], FP32, tag=f"mn4{lvl}{i}{j}",
                                  name=f"mn4{lvl}{i}{j}")
                    nc.vector.tensor_tensor(mn[:], X[:], Y[:], ALU.min)
                    dm = crf.tile([1, curW], FP32, tag=f"dm4{lvl}{i}{j}",
                                  name=f"dm4{lvl}{i}{j}")
                    nc.vector.tensor_tensor(dm[:], mn[:], M[:], ALU.subtract)
                    spe = crf.tile([1, curW], FP32, tag=f"spe4{lvl}{i}{j}",
                                   name=f"spe4{lvl}{i}{j}")
                    nc.scalar.activation(spe[:], dm[:], AF.Exp)
                    sp = crf.tile([1, curW], FP32, tag=f"sp4{lvl}{i}{j}",
                                  name=f"sp4{lvl}{i}{j}")
                    nc.scalar.activation(sp[:], spe[:], AF.Ln, bias=1.0)
                    nc.vector.tensor_tensor(nxtp[i][j][:], M[:], sp[:], ALU.add)
            curp = nxtp
            lvl += 1

        sc = crf.tile([1, 16], FP32, tag="scratch")

        def s_op(dst, a, b, op):
            nc.vector.tensor_tensor(dst, a, b, op)

        a0_ = sc[0:1, 0:1]; a1_ = sc[0:1, 1:2]
        s_op(a0_, curp[0][0][0:1, 0:1], tend_sb[0:1, 0:1], ALU.add)
        s_op(a1_, curp[1][0][0:1, 0:1], tend_sb[0:1, 1:2], ALU.add)
        M_ = sc[0:1, 2:3]; mn_ = sc[0:1, 3:4]; dm_ = sc[0:1, 4:5]; sp_ = sc[0:1, 5:6]
        s_op(M_, a0_, a1_, ALU.max)
        s_op(mn_, a0_, a1_, ALU.min)
        s_op(dm_, mn_, M_, ALU.subtract)
        spe_ = sc[0:1, 13:14]
        nc.scalar.activation(spe_, dm_, AF.Exp)
        nc.scalar.activation(sp_, spe_, AF.Ln, bias=1.0)
        logz = sc[0:1, 6:7]
        s_op(logz, M_, sp_, ALU.add)

        # ---- gold ----
        gsc = crf.tile([128, FW], FP32, tag="goldscratch")
        parts = crf.tile([128, 8], FP32, tag="parts")
        nc.vector.memset(parts[:], 0.0)
        ge = crf.tile([128, FW], FP32, tag="ge")
        nc.vector.tensor_tensor(ge[:], ep[1][:], ep[0][:], ALU.subtract)
        nc.vector.reduce_sum(parts[:, 0:1], ep[0][:], AX.X)
        nc.vector.scalar_tensor_tensor(gsc[:], ge[:], 1.0, lab_sb[:], ALU.mult,
                                       ALU.mult, accum_out=parts[:, 1:2])
        nc.vector.reduce_sum(parts[:, 2:3], laba_sb[:], AX.X)
        nc.vector.reduce_sum(parts[:, 3:4], labb_sb[:], AX.X)
        nc.vector.scalar_tensor_tensor(gsc[:], laba_sb[:], 1.0, labb_sb[:],
                                       ALU.mult, ALU.mult,
                                       accum_out=parts[:, 4:5])
        sums_ps = crfps.tile([1, 8], FP32, tag="sumsps")
        ones = crf.tile([128, 1], FP32, tag="ones")
        nc.vector.memset(ones[:], 1.0)
        nc.tensor.matmul(sums_ps[:], ones[:], parts[:], start=True, stop=True)
        sums = crf.tile([1, 8], FP32, tag="sums")
        nc.vector.tensor_copy(sums[:], sums_ps[:])

        l0 = labends_sb[0:1, 0:1]
        llast = labends_sb[0:1, 1:2]
        dts = sc[0:1, 7:8]; m1 = sc[0:1, 8:9]; tstart_t = sc[0:1, 9:10]
        s_op(dts, tstart_sb[0:1, 1:2], tstart_sb[0:1, 0:1], ALU.subtract)
        s_op(m1, l0, dts, ALU.mult)
        s_op(tstart_t, m1, tstart_sb[0:1, 0:1], ALU.add)
        dte = sc[0:1, 10:11]; m2 = sc[0:1, 11:12]; tend_t = sc[0:1, 12:13]
        s_op(dte, tend_sb[0:1, 1:2], tend_sb[0:1, 0:1], ALU.subtract)
        s_op(m2, llast, dte, ALU.mult)
        s_op(tend_t, m2, tend_sb[0:1, 0:1], ALU.add)

        sc2 = crf.tile([1, 16], FP32, tag="scratch2")
        dA = sc2[0:1, 0:1]; dB = sc2[0:1, 1:2]; dAB = sc2[0:1, 2:3]
        e1 = sc2[0:1, 3:4]
        s_op(dA, ttrans_sb[0:1, 2:3], ttrans_sb[0:1, 0:1], ALU.subtract)
        s_op(dB, ttrans_sb[0:1, 1:2], ttrans_sb[0:1, 0:1], ALU.subtract)
        s_op(e1, ttrans_sb[0:1, 3:4], ttrans_sb[0:1, 2:3], ALU.subtract)
        s_op(dAB, e1, dB, ALU.subtract)
        t00s = sc2[0:1, 4:5]
        nc.scalar.mul(t00s, ttrans_sb[0:1, 0:1], float(T - 1))
        tA = sc2[0:1, 5:6]; tB = sc2[0:1, 6:7]; tAB = sc2[0:1, 7:8]
        s_op(tA, sums[0:1, 2:3], dA, ALU.mult)
        s_op(tB, sums[0:1, 3:4], dB, ALU.mult)
        s_op(tAB, sums[0:1, 4:5], dAB, ALU.mult)
        acc1 = sc2[0:1, 8:9]; acc2 = sc2[0:1, 9:10]; acc3 = sc2[0:1, 10:11]
        s_op(acc1, t00s, tA, ALU.add)
        s_op(acc2, acc1, tB, ALU.add)
        s_op(acc3, acc2, tAB, ALU.add)
        g1 = sc2[0:1, 11:12]; g2 = sc2[0:1, 12:13]; g3 = sc2[0:1, 13:14]
        g4 = sc2[0:1, 14:15]
        s_op(g1, tstart_t, sums[0:1, 0:1], ALU.add)
        s_op(g2, g1, sums[0:1, 1:2], ALU.add)
        s_op(g3, g2, acc3, ALU.add)
        s_op(g4, g3, tend_t, ALU.add)
        res = sc2[0:1, 15:16]
        s_op(res, g4, logz, ALU.subtract)
        nc.sync.dma_start(out_d[0:1, 0:1], res)
        crfps.release()
        crf.release()
    else:
        nc.sync.dma_start(out_d[0:1, 0:1], tstart_sb[0:1, 0:1])
    att.release()
    dram.release()
    pers.release()


def build_program(cfg: Cfg):
    nc = bacc.Bacc("TRN2", target_bir_lowering=False, debug=False,
                   num_devices=cfg.NC)
    with tile.TileContext(nc) as tc:
        build(nc, tc, cfg)
    nc.compile()
    return nc


# ============================================================================
# Harness entry point
# ============================================================================

_CACHE = {}


def _get_program(cfg_key, cfg):
    if cfg_key not in _CACHE:
        _CACHE[cfg_key] = build_program(cfg)
    return _CACHE[cfg_key]


def kernel(**inputs):
    """Full-input kernel: shards across 8 NeuronCores internally."""
    from concourse import bass_utils

    bhh_zero = all(
        not np.any(np.asarray(inputs[f"{enc}_bhh_{dr}"])[2 * 512:])
        for enc in ("src", "tgt") for dr in ("f", "b"))
    cfg = Cfg(T=2048, U=32, W=32, skip_bhhn=bhh_zero)
    nc = _get_program(("main", bhh_zero), cfg)
    in_maps = prep_in_maps(inputs, cfg)
    res = bass_utils.run_bass_kernel_spmd(
        nc, in_maps, core_ids=list(range(cfg.NC)))
    out = np.asarray(res.results[0]["out_scalar"], dtype=np.float32)
    return out.reshape(())


# revision 5
# speedup vs baseline: 1.1355x; 1.1355x over previous
"""Trainium2 Bass kernel for nn_EstimatorCRF: BiGRU x2 -> cross/self attention -> emit -> CRF.

v2: sequence-parallel GRU. Each direction's scan is split into NSEG=4 segments
with a W-step warmup (GRU state decays fast, warmup is exact for seg 0 via
zero-forced gates and near-exact elsewhere since warmup gx is the true gx of
the preceding W steps, computed on host). 8 cores = 4 directions x 2 core
halves; each core runs CH=NSEG/2 independent chains interleaved in one
hardware loop so cross-engine latency of one chain hides under the other.

  - AG #1: feat-major [feat, t] blocks (per-core: CH segments side by side).
  - AG #2: natural [t, feat] bf16 blocks.
  - attention/emit q-sharded (QB = T/8 rows per core), K cached in SBUF.
  - AG #3: emit rows; CRF + gold replicated; scalar output.
"""

import sys
for _p in ("/opt/trn_rl_repo",):
    if _p not in sys.path:
        sys.path.insert(0, _p)


import numpy as np
import ml_dtypes

import concourse.bass as bass
import concourse.mybir as mybir
import concourse.tile as tile
from concourse import bacc
from concourse.bass import ds, ts
from concourse.masks import make_identity

FP32 = mybir.dt.float32
BF16 = mybir.dt.bfloat16
AF = mybir.ActivationFunctionType
ALU = mybir.AluOpType
AX = mybir.AxisListType

NEG_BIG = -1.0e30


class Cfg:
    def __init__(self, T=2048, E=512, H=512, U=32, n_cores=8, NSEG=4, W=32,
                 debug_outs=False, skip_bhhn=False, stop_after=None):
        self.T, self.E, self.H, self.U, self.NC = T, E, H, U, n_cores
        self.NSEG = NSEG
        self.CH = NSEG // 2           # chains per core
        self.SW = T // NSEG           # real steps per chain
        self.W = W                    # warmup steps per chain
        self.Tsc = self.SW + W        # total scan steps per chain
        self.debug_outs = debug_outs
        self.skip_bhhn = skip_bhhn
        self.stop_after = stop_after
        assert E == 512 and H == 512, "layout hardcoded for E=H=512"
        self.HC = H // 128            # h chunks (4)
        self.GC = 3 * self.HC         # gate chunks (12)
        self.G3 = 3 * H               # 1536
        self.QB = T // n_cores        # q rows per core
        assert self.QB % 128 == 0
        self.QTN = self.QB // 128
        self.FW = T // 128
        assert self.Tsc % U == 0
        assert self.SW % 128 == 0


# ----------------------------------------------------------------------------
# Host-side input preparation
# ----------------------------------------------------------------------------

def prep_in_maps(inputs, cfg: Cfg):
    T, H, E = cfg.T, cfg.H, cfg.E
    CH, SW, W, GC = cfg.CH, cfg.SW, cfg.W, cfg.GC
    f32 = np.float32
    bf16 = ml_dtypes.bfloat16
    d = {k: np.asarray(v) for k, v in inputs.items()}

    dirs = []  # (x [T,E] scan-ordered, rev, Wih, Whh, bih, bhh)
    for enc, xname in (("src", "source"), ("tgt", "target")):
        x = d[xname][0]
        for dr, rev in (("f", False), ("b", True)):
            xs = x[::-1] if rev else x
            dirs.append((xs, rev, d[f"{enc}_Wih_{dr}"], d[f"{enc}_Whh_{dr}"],
                         d[f"{enc}_bih_{dr}"], d[f"{enc}_bhh_{dr}"]))

    labels = np.asarray(d["labels"]).astype(np.int64)
    lab = labels.astype(f32)
    FW = cfg.FW

    def tplane(v):
        return np.ascontiguousarray(v.reshape(128, FW).astype(f32))

    laba = np.zeros(T, f32); laba[:T - 1] = lab[1:]
    labb = np.zeros(T, f32); labb[:T - 1] = lab[:T - 1]

    t_trans = d["t_trans"].astype(f32)
    ttrans_b = np.tile(t_trans.reshape(1, 4), (128, 1)).astype(f32)
    tstart_b = np.tile(d["t_start"].reshape(1, 2), (128, 1)).astype(f32)
    tend_b = np.tile(d["t_end"].reshape(1, 2), (128, 1)).astype(f32)
    wemitT = np.ascontiguousarray(d["W_emit"].astype(f32).T).astype(bf16)
    bemit = np.tile(d["b_emit"].reshape(1, 2), (128, 1)).astype(f32)

    in_maps = []
    NSEG = cfg.NSEG
    for c in range(cfg.NC):
        xs, rev, Wih, Whh, bih, bhh = dirs[c % 4]
        wihT = np.ascontiguousarray(Wih.astype(f32).T)         # [E, 3H]
        whhT = np.ascontiguousarray(Whh.astype(f32).T).astype(bf16)
        gxb = bih.astype(f32).copy()
        gxb[:2 * H] += bhh[:2 * H].astype(f32)                 # fold bhh_{r,z}
        gxbias = np.ascontiguousarray(gxb.reshape(GC, 128).T)  # [128, GC]
        bhhn = np.ascontiguousarray(bhh[2 * H:].astype(f32).reshape(cfg.HC, 128).T)

        # per-chain real inputs [E, CH*SW] and warmup gx [128, CH*GC*W]
        xT = np.zeros((E, CH * SW), f32)  # converted to bf16 below
        gxw = np.zeros((128, CH * GC * W), f32)
        for j in range(CH):
            sg = CH * (c // 4) + j            # true segment this chain owns
            s = (NSEG - 1 - sg) if rev else sg  # scan-order segment
            xT[:, j * SW:(j + 1) * SW] = xs[s * SW:(s + 1) * SW].T
            if s == 0:
                g = np.zeros((W, 3 * H), f32)
                g[:, H:2 * H] = 30.0       # z ~= 1 and gxn = 0: h stays 0
            else:
                g = xs[s * SW - W:s * SW].astype(f32) @ wihT + gxb  # [W, 3H]
            # [W, 3H] -> [128, GC, W]  (gate g3 = cchunk*128 + p)
            gw = np.ascontiguousarray(g.T.reshape(GC, 128, W).transpose(1, 0, 2))
            gxw[:, j * GC * W:(j + 1) * GC * W] = gw.reshape(128, GC * W)

        qoff = c * cfg.QB
        dm = np.zeros((cfg.QB, T), f32)
        for i in range(cfg.QB):
            dm[i, qoff + i] = NEG_BIG

        in_maps.append(dict(
            xT=xT.astype(bf16), wihT=wihT.astype(bf16), whhT=whhT,
            gxbias=gxbias, bhhn=bhhn,
            gxwarm=gxw.astype(bf16),
            diagmask=dm, wemitT=wemitT, bemit=bemit,
            ttrans_b=ttrans_b, tstart_b=tstart_b, tend_b=tend_b,
            lab16=tplane(lab), laba16=tplane(laba), labb16=tplane(labb),
            labends=np.tile(np.array([[lab[0], lab[T - 1]]], f32), (128, 1)),
        ))
    return in_maps


# ----------------------------------------------------------------------------
# Kernel builder
# ----------------------------------------------------------------------------

def build(nc: bacc.Bacc, tc: tile.TileContext, cfg: Cfg):
    T, E, H, U = cfg.T, cfg.E, cfg.H, cfg.U
    HC, GC, G3, QB, QTN, FW = cfg.HC, cfg.GC, cfg.G3, cfg.QB, cfg.QTN, cfg.FW
    CH, SW, W, Tsc, NSEG = cfg.CH, cfg.SW, cfg.W, cfg.Tsc, cfg.NSEG
    Tsc1 = Tsc + 1
    NK = E // 128
    NC = cfg.NC
    _ORDER = ["none", "gx", "scan", "ag", "attnq", "attnk", "attn",
              "emit", "ag3", None]
    lim = _ORDER.index(cfg.stop_after)

    def din(name, shape, dt=FP32):
        return nc.dram_tensor(name, list(shape), dt, kind="ExternalInput")

    xT_d = din("xT", (E, CH * SW), BF16)
    wihT_d = din("wihT", (E, G3), BF16)
    whhT_d = din("whhT", (H, G3), BF16)
    gxbias_d = din("gxbias", (128, GC))
    bhhn_d = din("bhhn", (128, HC))
    gxwarm_d = din("gxwarm", (128, CH * GC * W), BF16)
    diag_d = din("diagmask", (QB, T))
    wemitT_d = din("wemitT", (6 * H, 2), BF16)
    bemit_d = din("bemit", (128, 2))
    ttrans_d = din("ttrans_b", (128, 4))
    tstart_d = din("tstart_b", (128, 2))
    tend_d = din("tend_b", (128, 2))
    lab_d = din("lab16", (128, FW))
    laba_d = din("laba16", (128, FW))
    labb_d = din("labb16", (128, FW))
    labends_d = din("labends", (128, 2))

    out_d = nc.dram_tensor("out_scalar", [1, 1], FP32, kind="ExternalOutput")
    if cfg.debug_outs:
        dbg_emit_d = nc.dram_tensor("dbg_emit", [T, 2], FP32, kind="ExternalOutput")
        dbg_ys_d = nc.dram_tensor("dbg_ys", [CH * 2 * HC * 128, Tsc1], FP32,
                                  kind="ExternalOutput")

    pid = nc.partition_id()
    sel = pid % 2              # 1 on backward-direction cores

    # static map: true segment sg of direction dd -> (ag block, col base)
    # (host assigns bwd chains reversed segments, so this is dir-independent)
    def seg_src(dd, sg):
        return dd + 4 * (sg // CH), (sg % CH) * SW

    # ---- persistent small SBUF ----
    pers = tc.alloc_tile_pool(name="pers", bufs=1)
    whh_sb = pers.tile([128, HC * G3], BF16, tag="whh")
    gxbias_sb = pers.tile([128, GC], FP32, tag="gxbias")
    bhhn_sb = pers.tile([128, HC], FP32, tag="bhhn")
    ident = pers.tile([128, 128], FP32, tag="ident")
    make_identity(nc, ident[:])
    ident_bf = pers.tile([128, 128], BF16, tag="identbf")
    nc.vector.tensor_copy(ident_bf[:], ident[:])
    ttrans_sb = pers.tile([128, 4], FP32, tag="ttr")
    tstart_sb = pers.tile([128, 2], FP32, tag="tst")
    tend_sb = pers.tile([128, 2], FP32, tag="ten")
    lab_sb = pers.tile([128, FW], FP32, tag="lab")
    laba_sb = pers.tile([128, FW], FP32, tag="laba")
    labb_sb = pers.tile([128, FW], FP32, tag="labb")
    bemit_sb = pers.tile([128, 2], FP32, tag="bemit")
    wemit_sb = pers.tile([128, (6 * H // 128) * 2], BF16, tag="wemit")
    labends_sb = pers.tile([128, 2], FP32, tag="labends")

    nc.sync.dma_start(whh_sb[:], whhT_d.ap().rearrange("(k p) g -> p k g", p=128))
    nc.sync.dma_start(gxbias_sb[:], gxbias_d[:, :])
    nc.sync.dma_start(bhhn_sb[:], bhhn_d[:, :])
    nc.sync.dma_start(ttrans_sb[:], ttrans_d[:, :])
    nc.sync.dma_start(tstart_sb[:], tstart_d[:, :])
    nc.sync.dma_start(tend_sb[:], tend_d[:, :])
    nc.sync.dma_start(lab_sb[:], lab_d[:, :])
    nc.sync.dma_start(laba_sb[:], laba_d[:, :])
    nc.sync.dma_start(labb_sb[:], labb_d[:, :])
    nc.sync.dma_start(bemit_sb[:], bemit_d[:, :])
    nc.sync.dma_start(labends_sb[:], labends_d[:, :])
    nc.sync.dma_start(wemit_sb[:], wemitT_d.ap().rearrange("(k p) c -> p k c", p=128))

    # ---- DRAM pools for collectives ----
    dram = tc.alloc_tile_pool(name="dram", bufs=1, space="DRAM")
    b1_in = dram.tile([HC * 128, CH * SW], BF16, tag="b1i")
    ag1 = dram.tile([NC * HC * 128, CH * SW], BF16, tag="ag1", addr_space="Shared")
    b3_in = dram.tile([QB, 2], FP32, tag="b3i")
    ag3 = dram.tile([NC * QB, 2], FP32, tag="ag3", addr_space="Shared")

    ys_pool = tc.alloc_tile_pool(name="ysp", bufs=1)
    ys = ys_pool.tile([128, CH * 2 * HC * Tsc1], FP32, tag="ys")
    ysv = ys[:].rearrange("p (a t) -> p a t", a=CH * 2 * HC)

    # ============================ phase 1 + 2 =================================
    with tc.tile_pool(name="gxp", bufs=1) as gxp:
        gx_sb = gxp.tile([128, CH * GC * Tsc], BF16, tag="gx")
        gxv = gx_sb[:].rearrange("p (c t) -> p c t", c=CH * GC)

        # warmup gx columns from host
        nc.sync.dma_start(gxv[:, :, 0:W],
                          gxwarm_d.ap().rearrange("p (c w) -> p c w", c=CH * GC))

        if lim >= 1:
            with tc.tile_pool(name="ph1", bufs=1) as ph1, \
                 tc.tile_pool(name="ph1ps", bufs=2, space="PSUM") as ph1ps:
                xT_sb = ph1.tile([128, NK * CH * SW], BF16, tag="xT")
                wih_sb = ph1.tile([128, NK * G3], BF16, tag="wih")
                nc.sync.dma_start(
                    xT_sb[:], xT_d.ap().rearrange("(k p) t -> p k t", p=128))
                nc.sync.dma_start(
                    wih_sb[:], wihT_d.ap().rearrange("(k p) g -> p k g", p=128))

                CHK = min(SW, 512)
                for ch in range(CH):
                    for c in range(GC):
                        for n in range(SW // CHK):
                            ps = ph1ps.tile([128, CHK], FP32, tag="gxps")
                            for k in range(NK):
                                nc.tensor.matmul(
                                    ps[:, :],
                                    wih_sb[:, k * G3 + c * 128:
                                           k * G3 + (c + 1) * 128],
                                    xT_sb[:, k * CH * SW + ch * SW + n * CHK:
                                          k * CH * SW + ch * SW + (n + 1) * CHK],
                                    start=(k == 0), stop=(k == NK - 1))
                            nc.vector.tensor_scalar_add(
                                gxv[:, ch * GC + c,
                                    W + n * CHK:W + (n + 1) * CHK],
                                ps[:, :], gxbias_sb[:, c:c + 1])

        # ---- GRU scan: CH interleaved chains ----
        if lim >= 2:
            for ch in range(CH):
                nc.vector.memset(ysv[:, ch * 2 * HC:(ch + 1) * 2 * HC, 0:1], 0.0)
            skip_bhhn = cfg.skip_bhhn
            with tc.tile_pool(name="scan", bufs=3) as scp, \
                 tc.tile_pool(name="scanps0", bufs=2, space="PSUM") as pspl0, \
                 tc.tile_pool(name="scanps1", bufs=2, space="PSUM") as pspl1, \
                 tc.tile_pool(name="scanps2", bufs=2, space="PSUM") as pspl2, \
                 tc.tile_pool(name="scanps3", bufs=2, space="PSUM") as pspl3:
                pspl = [pspl0, pspl1, pspl2, pspl3][:CH]
                with tc.For_i(0, Tsc, U, staggered_reset=True,
                              hint_engines=(mybir.EngineType.PE,
                                            mybir.EngineType.DVE)) as iv:
                    hbf_prev = [None] * CH
                    for u in range(U):
                        j = iv + u
                        A0 = [ch * 2 * HC for ch in range(CH)]
                        G0 = [ch * GC for ch in range(CH)]
                        hbf = []
                        for ch in range(CH):
                            if u > 0:
                                hbf.append(hbf_prev[ch])
                            else:
                                h = scp.tile([128, HC], BF16, tag=f"hbf{ch}")
                                nc.vector.tensor_copy(
                                    h[:], ysv[:, A0[ch]:A0[ch] + HC, ds(j, 1)])
                                hbf.append(h)
                        pAB = [pspl[ch].tile([128, 12], FP32, tag=f"pAB{ch}",
                                             name=f"pAB{ch}")
                               for ch in range(CH)]
                        pA = [t[:, 0:8] for t in pAB]
                        pB = [t[:, 8:12] for t in pAB]
                        # r-gate matmuls, both chains
                        for ch in range(CH):
                            for c in range(0, 4):
                                for k in range(HC):
                                    nc.tensor.matmul(
                                        pA[ch][:, c:c + 1],
                                        whh_sb[:, k * G3 + c * 128:
                                               k * G3 + (c + 1) * 128],
                                        hbf[ch][:, k:k + 1], start=(k == 0),
                                        stop=False)
                                nc.tensor.matmul(
                                    pA[ch][:, c:c + 1], ident_bf[:],
                                    gxv[:, G0[ch] + c, ds(j, 1)],
                                    start=False, stop=True)
                        sr = [scp.tile([128, 4], FP32, tag=f"sr{ch}", name=f"sr{ch}")
                              for ch in range(CH)]
                        for ch in range(CH):
                            nc.scalar.activation(sr[ch][:], pA[ch][:, 0:4],
                                                 AF.Sigmoid)
                        # n- and z-gate matmuls, both chains
                        for ch in range(CH):
                            for c in range(8, 12):
                                for k in range(HC):
                                    nc.tensor.matmul(
                                        pB[ch][:, c - 8:c - 7],
                                        whh_sb[:, k * G3 + c * 128:
                                               k * G3 + (c + 1) * 128],
                                        hbf[ch][:, k:k + 1], start=(k == 0),
                                        stop=(k == HC - 1))
                            for c in range(4, 8):
                                for k in range(HC):
                                    nc.tensor.matmul(
                                        pA[ch][:, c:c + 1],
                                        whh_sb[:, k * G3 + c * 128:
                                               k * G3 + (c + 1) * 128],
                                        hbf[ch][:, k:k + 1], start=(k == 0),
                                        stop=False)
                                nc.tensor.matmul(
                                    pA[ch][:, c:c + 1], ident_bf[:],
                                    gxv[:, G0[ch] + c, ds(j, 1)],
                                    start=False, stop=True)
                        sz = [scp.tile([128, 4], FP32, tag=f"sz{ch}", name=f"sz{ch}")
                              for ch in range(CH)]
                        for ch in range(CH):
                            nc.scalar.activation(sz[ch][:], pA[ch][:, 4:8],
                                                 AF.Sigmoid)
                        tn2 = [scp.tile([128, HC], FP32, tag=f"tn2{ch}", name=f"tn2{ch}")
                               for ch in range(CH)]
                        tn3 = [scp.tile([128, HC], FP32, tag=f"tn3{ch}", name=f"tn3{ch}")
                               for ch in range(CH)]
                        for ch in range(CH):
                            if skip_bhhn:
                                nsrc = pB[ch][:, :]
                            else:
                                tn1 = scp.tile([128, HC], FP32, tag=f"tn1{ch}")
                                nc.vector.tensor_tensor(tn1[:], pB[ch][:, :],
                                                        bhhn_sb[:], ALU.add)
                                nsrc = tn1[:]
                            nc.vector.tensor_tensor(tn2[ch][:], nsrc, sr[ch][:],
                                                    ALU.mult)
                            nc.vector.tensor_tensor(
                                tn3[ch][:], tn2[ch][:],
                                gxv[:, G0[ch] + 8:G0[ch] + 12, ds(j, 1)], ALU.add)
                        nn = [scp.tile([128, HC], FP32, tag=f"nn{ch}", name=f"nn{ch}")
                              for ch in range(CH)]
                        for ch in range(CH):
                            nc.scalar.activation(nn[ch][:], tn3[ch][:], AF.Tanh)
                        zm1 = [scp.tile([128, 4], FP32, tag=f"zm1{ch}", name=f"zm1{ch}")
                               for ch in range(CH)]
                        t1 = [scp.tile([128, HC], FP32, tag=f"t1{ch}", name=f"t1{ch}")
                              for ch in range(CH)]
                        for ch in range(CH):
                            nc.gpsimd.tensor_scalar_sub(zm1[ch][:], sz[ch][:], 1.0)
                            nc.gpsimd.tensor_tensor(
                                t1[ch][:], sz[ch][:],
                                ysv[:, A0[ch]:A0[ch] + HC, ds(j, 1)], ALU.mult)
                        for ch in range(CH):
                            t2 = scp.tile([128, HC], FP32, tag=f"t2{ch}")
                            nc.vector.tensor_tensor(t2[:], nn[ch][:], zm1[ch][:],
                                                    ALU.mult)
                            hbf2 = scp.tile([128, HC], BF16, tag=f"hbf2{ch}")
                            nc.vector.tensor_tensor(hbf2[:], t1[ch][:], t2[:],
                                                    ALU.subtract)
                            nc.gpsimd.tensor_tensor(
                                ysv[:, A0[ch]:A0[ch] + HC, ds(j + 1, 1)],
                                t1[ch][:], t2[:], ALU.subtract)
                            nc.gpsimd.tensor_copy(
                                ysv[:, A0[ch] + HC:A0[ch] + 2 * HC,
                                    ds((Tsc - u) - iv, 1)],
                                ysv[:, A0[ch]:A0[ch] + HC, ds(j + 1, 1)])
                            hbf_prev[ch] = hbf2

    if cfg.debug_outs:
        nc.sync.dma_start(dbg_ys_d.ap().rearrange("(a p) t -> p a t", p=128),
                          ysv[:, :, :])

    # =================== phase 3: natural transposes + AGs ===================
    if lim >= 3:
        natp = tc.alloc_tile_pool(name="natp", bufs=1)
        conv = natp.tile([128, CH * HC * SW], BF16, tag="conv")
        for ch in range(CH):
            a0 = ch * 2 * HC
            nc.vector.tensor_copy(
                conv[:].rearrange("p (c k t) -> p c k t", c=CH, k=HC)[:, ch, :, :],
                ysv[:, ds(a0 + sel * HC, HC), ds(1 + W - W * sel, SW)])
        for ch in range(CH):
            nc.gpsimd.dma_start(
                b1_in[:, ch * SW:(ch + 1) * SW]
                .rearrange("(k p) t -> p k t", p=128),
                conv[:].rearrange("p (c k t) -> p c k t", c=CH, k=HC)[:, ch, :, :])
        nc.gpsimd.collective_compute(
            "AllGather", ALU.bypass, ins=[b1_in.opt()], outs=[ag1.opt()],
            replica_groups=[list(range(NC))])

        natp.release()
    ys_pool.release()

    # ===================== phase 4: attention (q-sharded) =====================
    att = tc.alloc_tile_pool(name="att", bufs=1)
    qt_sb = att.tile([128, 8 * QB], BF16, tag="qt")
    qt_cand = att.tile([128, 2 * 8 * QB], BF16, tag="qtc")
    diag_sb = att.tile([128, QTN * T], FP32, tag="diag")
    featsT = att.tile([128, 24 * QB], BF16, tag="featsT")
    pt_sb = att.tile([128, (T // 128) * QB], BF16, tag="ptq")
    natfull = att.tile([128, (T // 128) * 1024], BF16, tag="natfull")
    kfull = att.tile([128, 8 * T], BF16, tag="kfull")
    emit_sb = att.tile([128, QTN * 2], FP32, tag="emit")

    if lim >= 4:
        # queries: rows [pid*QB, pid*QB+QB) of tgt features, runtime-stitched
        ratio = SW // QB                       # power of 2
        sh = ratio.bit_length() - 1
        sgq = pid >> sh                        # true segment of this q range
        oq = (pid & (ratio - 1)) * QB          # offset within segment
        cb = nc.s_assert_within((sgq & (CH - 1)) * SW + oq, 0, CH * SW - QB,
                                skip_runtime_assert=True)
        # runtime rows on a Shared tile crash the DMA engine: load both core
        # halves' candidate blocks (static rows, runtime col) and select after
        for hh in range(2):
            nc.gpsimd.dma_start(
                qt_cand[:, hh * 8 * QB:(hh + 1) * 8 * QB]
                .rearrange("p (k q) -> p k q", k=8),
                ag1[(2 + 4 * hh) * 512:(4 + 4 * hh) * 512, ds(cb, QB)]
                .rearrange("(k p) t -> p k t", p=128))
        sgh = sgq >> (CH.bit_length() - 1)     # which core half owns it
        qoff = nc.s_assert_within(sgh * (8 * QB), 0, 8 * QB,
                                  skip_runtime_assert=True)
        nc.vector.tensor_copy(qt_sb[:], qt_cand[:, ds(qoff, 8 * QB)])
        nc.vector.tensor_copy(featsT[:, 0:8 * QB], qt_sb[:])
        nc.vector.tensor_scalar_mul(qt_sb[:], qt_sb[:], float(np.sqrt(2.0 * H)))
        nc.sync.dma_start(diag_sb[:].rearrange("p (q t) -> p q t", q=QTN),
                          diag_d.ap().rearrange("(q p) t -> p q t", p=128))

    with tc.tile_pool(name="psS", bufs=1, space="PSUM") as psS, \
         tc.tile_pool(name="psT", bufs=2, space="PSUM") as psT, \
         tc.tile_pool(name="psC", bufs=2, space="PSUM") as psC, \
         tc.tile_pool(name="Pp", bufs=2) as Pp, \
         tc.tile_pool(name="attsm", bufs=4) as attsm:
        if lim >= 5:
            ST = T // 128
            for at in range(2):
                if at == 1 and lim < 6:
                    continue
                kv0 = 0 if at == 0 else 2
                # K cache: [feat-chunk kt][true t] for this attention's 2 dirs
                for dpart in range(2):
                    for sg in range(NSEG):
                        blk, cb = seg_src(kv0 + dpart, sg)
                        nc.sync.dma_start(
                            kfull[:, :].rearrange("p (a t) -> p a t", a=8)
                            [:, dpart * NK:(dpart + 1) * NK, sg * SW:(sg + 1) * SW],
                            ag1[blk * 512:(blk + 1) * 512, cb:cb + SW]
                            .rearrange("(k p) t -> p k t", p=128))
                # V natural tiles: transpose the K cache on-core
                for st in range(ST):
                    for m in range(8):
                        kt = (m // NK) * NK + (m % NK)
                        ntp = psT.tile([128, 128], BF16, tag="ptp")
                        nc.tensor.transpose(
                            ntp[:], kfull[:, kt * T + st * 128:
                                          kt * T + (st + 1) * 128], ident_bf[:])
                        nc.vector.tensor_copy(
                            natfull[:, st * 1024 + m * 128:
                                    st * 1024 + (m + 1) * 128], ntp[:])
                if lim < 6:
                    continue
                for qi in range(QTN):
                    pS = [psS.tile([128, T // 2], FP32, tag=f"pS{sh}",
                                   name=f"pS{sh}") for sh in range(2)]
                    for kt in range(8):
                        for nch in range(T // 512):
                            sh = nch // (T // 1024)
                            off = (nch % (T // 1024)) * 512
                            nc.tensor.matmul(
                                pS[sh][:, off:off + 512],
                                qt_sb[:, kt * QB + qi * 128:
                                      kt * QB + (qi + 1) * 128],
                                kfull[:, kt * T + nch * 512:
                                      kt * T + (nch + 1) * 512],
                                start=(kt == 0), stop=(kt == 7))
                    if at == 1:
                        for sh in range(2):
                            nc.vector.tensor_tensor(
                                pS[sh][:, :], pS[sh][:, :],
                                diag_sb[:, qi * T + sh * (T // 2):
                                        qi * T + (sh + 1) * (T // 2)],
                                ALU.add)
                    mx = [attsm.tile([128, 1], FP32, tag=f"mx{sh}",
                                     name=f"mx{sh}") for sh in range(2)]
                    for sh in range(2):
                        nc.vector.reduce_max(mx[sh][:], pS[sh][:, :], AX.X)
                    negm = attsm.tile([128, 1], FP32, tag="negm")
                    nc.vector.tensor_tensor(negm[:], mx[0][:], mx[1][:], ALU.max)
                    nc.vector.tensor_scalar_mul(negm[:], negm[:], -1.0)
                    Pt = Pp.tile([128, T], FP32, tag="P")
                    Ptb = Pp.tile([128, T], BF16, tag="Pb")
                    sm = [attsm.tile([128, 1], FP32, tag=f"sm{sh}",
                                     name=f"sm{sh}") for sh in range(2)]
                    for sh in range(2):
                        nc.scalar.activation(
                            Pt[:, sh * (T // 2):(sh + 1) * (T // 2)],
                            pS[sh][:, :], AF.Exp, bias=negm[:],
                            accum_out=sm[sh][:])
                    smc = attsm.tile([128, 1], FP32, tag="smc")
                    nc.vector.tensor_tensor(smc[:], sm[0][:], sm[1][:], ALU.add)
                    rinv = attsm.tile([128, 1], FP32, tag="rinv")
                    nc.vector.reciprocal(rinv[:], smc[:])
                    nc.vector.tensor_scalar_mul(Ptb[:, :], Pt[:, :], rinv[:])
                    for st in range(ST):
                        ptp = psT.tile([128, 128], BF16, tag="ptp")
                        nc.tensor.transpose(ptp[:], Ptb[:, st * 128:(st + 1) * 128],
                                            ident_bf[:])
                        nc.vector.tensor_copy(
                            pt_sb[:, st * QB + qi * 128:
                                  st * QB + (qi + 1) * 128],
                            ptp[:])
                for m in range(8):
                    pc = psC.tile([128, QB], FP32, tag="pc")
                    for st in range(ST):
                        nc.tensor.matmul(
                            pc[:],
                            natfull[:, st * 1024 + m * 128:
                                    st * 1024 + (m + 1) * 128],
                            pt_sb[:, st * QB:(st + 1) * QB],
                            start=(st == 0), stop=(st == ST - 1))
                    nc.vector.tensor_copy(
                        featsT[:, (8 + at * 8 + m) * QB:(9 + at * 8 + m) * QB],
                        pc[:])

        if lim >= 7:
            for qi in range(QTN):
                pe = psC.tile([128, 2], FP32, tag="pc")
                for kt in range(24):
                    nc.tensor.matmul(
                        pe[:, :],
                        featsT[:, kt * QB + qi * 128: kt * QB + (qi + 1) * 128],
                        wemit_sb[:, kt * 2:(kt + 1) * 2],
                        start=(kt == 0), stop=(kt == 23))
                nc.vector.tensor_tensor(emit_sb[:, qi * 2:(qi + 1) * 2], pe[:, :],
                                        bemit_sb[:], ALU.add)

    if lim >= 8:
        nc.gpsimd.dma_start(b3_in[:].rearrange("(q p) c -> p q c", p=128),
                            emit_sb[:].rearrange("p (q c) -> p q c", q=QTN))
        nc.gpsimd.collective_compute(
            "AllGather", ALU.bypass, ins=[b3_in.opt()], outs=[ag3.opt()],
            replica_groups=[list(range(NC))])
    if cfg.debug_outs:
        nc.sync.dma_start(dbg_emit_d[:, :], ag3[0:T, :])

    # ========================= phase 5: CRF + gold ===========================
    if lim >= 9:
        crf = tc.alloc_tile_pool(name="crf", bufs=1)
        crfps = tc.alloc_tile_pool(name="crfps", bufs=2, space="PSUM")
        ep = [crf.tile([128, FW], FP32, tag=f"ep{i}", name=f"ep{i}")
              for i in range(2)]
        for i in range(2):
            nc.sync.dma_start(
                ep[i][:],
                ag3[0:T, :].rearrange("(p f) c -> p f c", p=128)[:, :, i:i + 1])

        pl = [[crf.tile([128, FW], FP32, tag=f"pl{i}{j}", name=f"pl{i}{j}")
               for j in range(2)] for i in range(2)]
        for i in range(2):
            for j in range(2):
                nc.vector.tensor_scalar_add(pl[i][j][:], ep[i][:],
                                            ttrans_sb[:, 2 * i + j: 2 * i + j + 1])
        for i in range(2):
            for j in range(2):
                nc.vector.tensor_tensor(pl[i][j][0:1, 0:1], ep[i][0:1, 0:1],
                                        tstart_sb[0:1, i:i + 1], ALU.add)

        cur = pl
        Wf = FW
        lvl = 0
        while Wf > 1:
            Wf //= 2
            nxt = [[crf.tile([128, Wf], FP32, tag=f"lv{lvl}_{i}{j}",
                             name=f"lv{lvl}_{i}{j}") for j in range(2)]
                   for i in range(2)]
            Aap = [[cur[k][j][:, 0:2 * Wf]
                    .rearrange("p (m two) -> p m two", two=2)[:, :, 0:1]
                    for j in range(2)] for k in range(2)]
            Bap = [[cur[i][k][:, 0:2 * Wf]
                    .rearrange("p (m two) -> p m two", two=2)[:, :, 1:2]
                    for k in range(2)] for i in range(2)]
            for i in range(2):
                for j in range(2):
                    X = crf.tile([128, Wf], FP32, tag=f"Xf{lvl}{i}{j}")
                    Y = crf.tile([128, Wf], FP32, tag=f"Yf{lvl}{i}{j}")
                    nc.vector.tensor_tensor(X[:], Bap[i][0], Aap[0][j], ALU.add)
                    nc.vector.tensor_tensor(Y[:], Bap[i][1], Aap[1][j], ALU.add)
                    M = crf.tile([128, Wf], FP32, tag=f"Mf{lvl}{i}{j}")
                    nc.vector.tensor_tensor(M[:], X[:], Y[:], ALU.max)
                    mn = crf.tile([128, Wf], FP32, tag=f"mnf{lvl}{i}{j}")
                    nc.vector.tensor_tensor(mn[:], X[:], Y[:], ALU.min)
                    dm = crf.tile([128, Wf], FP32, tag=f"dmf{lvl}{i}{j}")
                    nc.vector.tensor_tensor(dm[:], mn[:], M[:], ALU.subtract)
                    spe = crf.tile([128, Wf], FP32, tag=f"spef{lvl}{i}{j}")
                    nc.scalar.activation(spe[:], dm[:], AF.Exp)
                    sp = crf.tile([128, Wf], FP32, tag=f"spf{lvl}{i}{j}")
                    nc.scalar.activation(sp[:], spe[:], AF.Ln, bias=1.0)
                    nc.vector.tensor_tensor(nxt[i][j][:], M[:], sp[:], ALU.add)
            cur = nxt
            lvl += 1

        r128 = [[None, None], [None, None]]
        for i in range(2):
            for j in range(2):
                tps = crfps.tile([128, 128], FP32, tag="tps", name=f"tps{i}{j}")
                nc.tensor.transpose(tps[0:1, :], cur[i][j][:, 0:1], ident[:])
                rr = crf.tile([1, 128], FP32, tag=f"r128_{i}{j}",
                              name=f"r128_{i}{j}")
                nc.vector.tensor_copy(rr[:], tps[0:1, :])
                r128[i][j] = rr

        curp = r128
        curW = 128
        lvl = 0
        while curW > 1:
            curW //= 2
            nxtp = [[crf.tile([1, curW], FP32, tag=f"p4_{lvl}{i}{j}",
                              name=f"p4_{lvl}{i}{j}") for j in range(2)]
                    for i in range(2)]
            Aap = [[curp[k][j][0:1, 0:2 * curW]
                    .rearrange("p (m two) -> p m two", two=2)[:, :, 0:1]
                    for j in range(2)] for k in range(2)]
            Bap = [[curp[i][k][0:1, 0:2 * curW]
                    .rearrange("p (m two) -> p m two", two=2)[:, :, 1:2]
                    for k in range(2)] for i in range(2)]
            for i in range(2):
                for j in range(2):
                    X = crf.tile([1, curW], FP32, tag=f"X4{lvl}{i}{j}",
                                 name=f"X4{lvl}{i}{j}")
                    Y = crf.tile([1, curW], FP32, tag=f"Y4{lvl}{i}{j}",
                                 name=f"Y4{lvl}{i}{j}")
                    nc.vector.tensor_tensor(X[:], Bap[i][0], Aap[0][j], ALU.add)
                    nc.vector.tensor_tensor(Y[:], Bap[i][1], Aap[1][j], ALU.add)
                    M = crf.tile([1, curW], FP32, tag=f"M4{lvl}{i}{j}",
                                 name=f"M4{lvl}{i}{j}")
                    nc.vector.tensor_tensor(M[:], X[:], Y[:], ALU.max)
                    mn = crf.tile([1, curW], FP32, tag=f"mn4{lvl}{i}{j}",
                                  name=f"mn4{lvl}{i}{j}")
                    nc.vector.tensor_tensor(mn[:], X[:], Y[:], ALU.min)
                    dm = crf.tile([1, curW], FP32, tag=f"dm4{lvl}{i}{j}",
                                  name=f"dm4{lvl}{i}{j}")
                    nc.vector.tensor_tensor(dm[:], mn[:], M[:], ALU.subtract)
                    spe = crf.tile([1, curW], FP32, tag=f"spe4{lvl}{i}{j}",
                                   name=f"spe4{lvl}{i}{j}")
                    nc.scalar.activation(spe[:], dm[:], AF.Exp)
                    sp = crf.tile([1, curW], FP32, tag=f"sp4{lvl}{i}{j}",
                                  name=f"sp4{lvl}{i}{j}")
                    nc.scalar.activation(sp[:], spe[:], AF.Ln, bias=1.0)
                    nc.vector.tensor_tensor(nxtp[i][j][:], M[:], sp[:], ALU.add)
            curp = nxtp
            lvl += 1

        sc = crf.tile([1, 16], FP32, tag="scratch")

        def s_op(dst, a, b, op):
            nc.vector.tensor_tensor(dst, a, b, op)

        a0_ = sc[0:1, 0:1]; a1_ = sc[0:1, 1:2]
        s_op(a0_, curp[0][0][0:1, 0:1], tend_sb[0:1, 0:1], ALU.add)
        s_op(a1_, curp[1][0][0:1, 0:1], tend_sb[0:1, 1:2], ALU.add)
        M_ = sc[0:1, 2:3]; mn_ = sc[0:1, 3:4]; dm_ = sc[0:1, 4:5]; sp_ = sc[0:1, 5:6]
        s_op(M_, a0_, a1_, ALU.max)
        s_op(mn_, a0_, a1_, ALU.min)
        s_op(dm_, mn_, M_, ALU.subtract)
        spe_ = sc[0:1, 13:14]
        nc.scalar.activation(spe_, dm_, AF.Exp)
        nc.scalar.activation(sp_, spe_, AF.Ln, bias=1.0)
        logz = sc[0:1, 6:7]
        s_op(logz, M_, sp_, ALU.add)

        # ---- gold ----
        gsc = crf.tile([128, FW], FP32, tag="goldscratch")
        parts = crf.tile([128, 8], FP32, tag="parts")
        nc.vector.memset(parts[:], 0.0)
        ge = crf.tile([128, FW], FP32, tag="ge")
        nc.vector.tensor_tensor(ge[:], ep[1][:], ep[0][:], ALU.subtract)
        nc.vector.reduce_sum(parts[:, 0:1], ep[0][:], AX.X)
        nc.vector.scalar_tensor_tensor(gsc[:], ge[:], 1.0, lab_sb[:], ALU.mult,
                                       ALU.mult, accum_out=parts[:, 1:2])
        nc.vector.reduce_sum(parts[:, 2:3], laba_sb[:], AX.X)
        nc.vector.reduce_sum(parts[:, 3:4], labb_sb[:], AX.X)
        nc.vector.scalar_tensor_tensor(gsc[:], laba_sb[:], 1.0, labb_sb[:],
                                       ALU.mult, ALU.mult,
                                       accum_out=parts[:, 4:5])
        sums_ps = crfps.tile([1, 8], FP32, tag="sumsps")
        ones = crf.tile([128, 1], FP32, tag="ones")
        nc.vector.memset(ones[:], 1.0)
        nc.tensor.matmul(sums_ps[:], ones[:], parts[:], start=True, stop=True)
        sums = crf.tile([1, 8], FP32, tag="sums")
        nc.vector.tensor_copy(sums[:], sums_ps[:])

        l0 = labends_sb[0:1, 0:1]
        llast = labends_sb[0:1, 1:2]
        dts = sc[0:1, 7:8]; m1 = sc[0:1, 8:9]; tstart_t = sc[0:1, 9:10]
        s_op(dts, tstart_sb[0:1, 1:2], tstart_sb[0:1, 0:1], ALU.subtract)
        s_op(m1, l0, dts, ALU.mult)
        s_op(tstart_t, m1, tstart_sb[0:1, 0:1], ALU.add)
        dte = sc[0:1, 10:11]; m2 = sc[0:1, 11:12]; tend_t = sc[0:1, 12:13]
        s_op(dte, tend_sb[0:1, 1:2], tend_sb[0:1, 0:1], ALU.subtract)
        s_op(m2, llast, dte, ALU.mult)
        s_op(tend_t, m2, tend_sb[0:1, 0:1], ALU.add)

        sc2 = crf.tile([1, 16], FP32, tag="scratch2")
        dA = sc2[0:1, 0:1]; dB = sc2[0:1, 1:2]; dAB = sc2[0:1, 2:3]
        e1 = sc2[0:1, 3:4]
        s_op(dA, ttrans_sb[0:1, 2:3], ttrans_sb[0:1, 0:1], ALU.subtract)
        s_op(dB, ttrans_sb[0:1, 1:2], ttrans_sb[0:1, 0:1], ALU.subtract)
        s_op(e1, ttrans_sb[0:1, 3:4], ttrans_sb[0:1, 2:3], ALU.subtract)
        s_op(dAB, e1, dB, ALU.subtract)
        t00s = sc2[0:1, 4:5]
        nc.scalar.mul(t00s, ttrans_sb[0:1, 0:1], float(T - 1))
        tA = sc2[0:1, 5:6]; tB = sc2[0:1, 6:7]; tAB = sc2[0:1, 7:8]
        s_op(tA, sums[0:1, 2:3], dA, ALU.mult)
        s_op(tB, sums[0:1, 3:4], dB, ALU.mult)
        s_op(tAB, sums[0:1, 4:5], dAB, ALU.mult)
        acc1 = sc2[0:1, 8:9]; acc2 = sc2[0:1, 9:10]; acc3 = sc2[0:1, 10:11]
        s_op(acc1, t00s, tA, ALU.add)
        s_op(acc2, acc1, tB, ALU.add)
        s_op(acc3, acc2, tAB, ALU.add)
        g1 = sc2[0:1, 11:12]; g2 = sc2[0:1, 12:13]; g3 = sc2[0:1, 13:14]
        g4 = sc2[0:1, 14:15]
        s_op(g1, tstart_t, sums[0:1, 0:1], ALU.add)
        s_op(g2, g1, sums[0:1, 1:2], ALU.add)
        s_op(g3, g2, acc3, ALU.add)
        s_op(g4, g3, tend_t, ALU.add)
        res = sc2[0:1, 15:16]
        s_op(res, g4, logz, ALU.subtract)
        nc.sync.dma_start(out_d[0:1, 0:1], res)
        crfps.release()
        crf.release()
    else:
        nc.sync.dma_start(out_d[0:1, 0:1], tstart_sb[0:1, 0:1])
    att.release()
    dram.release()
    pers.release()


def build_program(cfg: Cfg):
    nc = bacc.Bacc("TRN2", target_bir_lowering=False, debug=False,
                   num_devices=cfg.NC)
    with tile.TileContext(nc) as tc:
        build(nc, tc, cfg)
    nc.compile()
    return nc


# ============================================================================
# Harness entry point
# ============================================================================

_CACHE = {}


def _get_program(cfg_key, cfg):
    if cfg_key not in _CACHE:
        _CACHE[cfg_key] = build_program(cfg)
    return _CACHE[cfg_key]


def kernel(**inputs):
    """Full-input kernel: shards across 8 NeuronCores internally."""
    from concourse import bass_utils

    bhh_zero = all(
        not np.any(np.asarray(inputs[f"{enc}_bhh_{dr}"])[2 * 512:])
        for enc in ("src", "tgt") for dr in ("f", "b"))
    cfg = Cfg(T=2048, U=32, W=32, skip_bhhn=bhh_zero)
    nc = _get_program(("main", bhh_zero), cfg)
    in_maps = prep_in_maps(inputs, cfg)
    res = bass_utils.run_bass_kernel_spmd(
        nc, in_maps, core_ids=list(range(cfg.NC)))
    out = np.asarray(res.results[0]["out_scalar"], dtype=np.float32)
    return out.reshape(())


# revision 6
# speedup vs baseline: 2.6578x; 2.3406x over previous
"""Trainium2 Bass kernel for nn_EstimatorCRF: BiGRU x2 -> cross/self attention -> emit -> CRF.

v2: sequence-parallel GRU. Each direction's scan is split into NSEG=4 segments
with a W-step warmup (GRU state decays fast, warmup is exact for seg 0 via
zero-forced gates and near-exact elsewhere since warmup gx is the true gx of
the preceding W steps, computed on host). 8 cores = 4 directions x 2 core
halves; each core runs CH=NSEG/2 independent chains interleaved in one
hardware loop so cross-engine latency of one chain hides under the other.

  - AG #1: feat-major [feat, t] blocks (per-core: CH segments side by side).
  - AG #2: natural [t, feat] bf16 blocks.
  - attention/emit q-sharded (QB = T/8 rows per core), K cached in SBUF.
  - AG #3: emit rows; CRF + gold replicated; scalar output.
"""

import sys
for _p in ("/opt/trn_rl_repo",):
    if _p not in sys.path:
        sys.path.insert(0, _p)


import numpy as np
import ml_dtypes

import concourse.bass as bass
import concourse.mybir as mybir
import concourse.tile as tile
from concourse import bacc
from concourse.bass import ds, ts
from concourse.masks import make_identity

FP32 = mybir.dt.float32
BF16 = mybir.dt.bfloat16
AF = mybir.ActivationFunctionType
ALU = mybir.AluOpType
AX = mybir.AxisListType

NEG_BIG = -1.0e30


class Cfg:
    def __init__(self, T=2048, E=512, H=512, U=32, n_cores=8, NSEG=4, W=32,
                 debug_outs=False, skip_bhhn=False, stop_after=None):
        self.T, self.E, self.H, self.U, self.NC = T, E, H, U, n_cores
        self.NSEG = NSEG
        self.CH = NSEG // 2           # chains per core
        self.SW = T // NSEG           # real steps per chain
        self.W = W                    # warmup steps per chain
        self.Tsc = self.SW + W        # total scan steps per chain
        self.debug_outs = debug_outs
        self.skip_bhhn = skip_bhhn
        self.stop_after = stop_after
        assert E == 512 and H == 512, "layout hardcoded for E=H=512"
        self.HC = H // 128            # h chunks (4)
        self.GC = 3 * self.HC         # gate chunks (12)
        self.G3 = 3 * H               # 1536
        self.QB = T // n_cores        # q rows per core
        assert self.QB % 128 == 0
        self.QTN = self.QB // 128
        self.FW = T // 128
        assert self.Tsc % U == 0
        assert self.SW % 128 == 0


# ----------------------------------------------------------------------------
# Host-side input preparation
# ----------------------------------------------------------------------------

def prep_in_maps(inputs, cfg: Cfg):
    T, H, E = cfg.T, cfg.H, cfg.E
    CH, SW, W, GC = cfg.CH, cfg.SW, cfg.W, cfg.GC
    f32 = np.float32
    bf16 = ml_dtypes.bfloat16
    d = {k: np.asarray(v) for k, v in inputs.items()}

    dirs = []  # (x [T,E] scan-ordered, rev, Wih, Whh, bih, bhh)
    for enc, xname in (("src", "source"), ("tgt", "target")):
        x = d[xname][0]
        for dr, rev in (("f", False), ("b", True)):
            xs = x[::-1] if rev else x
            dirs.append((xs, rev, d[f"{enc}_Wih_{dr}"], d[f"{enc}_Whh_{dr}"],
                         d[f"{enc}_bih_{dr}"], d[f"{enc}_bhh_{dr}"]))

    labels = np.asarray(d["labels"]).astype(np.int64)
    lab = labels.astype(f32)
    FW = cfg.FW

    def tplane(v):
        return np.ascontiguousarray(v.reshape(128, FW).astype(f32))

    laba = np.zeros(T, f32); laba[:T - 1] = lab[1:]
    labb = np.zeros(T, f32); labb[:T - 1] = lab[:T - 1]

    t_trans = d["t_trans"].astype(f32)
    ttrans_b = np.tile(t_trans.reshape(1, 4), (128, 1)).astype(f32)
    tstart_b = np.tile(d["t_start"].reshape(1, 2), (128, 1)).astype(f32)
    tend_b = np.tile(d["t_end"].reshape(1, 2), (128, 1)).astype(f32)
    wemitT = np.ascontiguousarray(d["W_emit"].astype(f32).T).astype(bf16)
    bemit = np.tile(d["b_emit"].reshape(1, 2), (128, 1)).astype(f32)

    in_maps = []
    NSEG = cfg.NSEG
    for c in range(cfg.NC):
        xs, rev, Wih, Whh, bih, bhh = dirs[c % 4]
        wihT = np.ascontiguousarray(Wih.astype(f32).T)         # [E, 3H]
        whhT = np.ascontiguousarray(Whh.astype(f32).T).astype(bf16)
        gxb = bih.astype(f32).copy()
        gxb[:2 * H] += bhh[:2 * H].astype(f32)                 # fold bhh_{r,z}
        gxbias = np.ascontiguousarray(gxb.reshape(GC, 128).T)  # [128, GC]
        bhhn = np.ascontiguousarray(bhh[2 * H:].astype(f32).reshape(cfg.HC, 128).T)

        # per-chain real inputs [E, CH*SW] and warmup gx [128, CH*GC*W]
        xT = np.zeros((E, CH * SW), f32)  # converted to bf16 below
        gxw = np.zeros((128, CH * GC * W), f32)
        for j in range(CH):
            sg = CH * (c // 4) + j            # true segment this chain owns
            s = (NSEG - 1 - sg) if rev else sg  # scan-order segment
            xT[:, j * SW:(j + 1) * SW] = xs[s * SW:(s + 1) * SW].T
            if s == 0:
                g = np.zeros((W, 3 * H), f32)
                g[:, H:2 * H] = 30.0       # z ~= 1 and gxn = 0: h stays 0
            else:
                g = xs[s * SW - W:s * SW].astype(f32) @ wihT + gxb  # [W, 3H]
            # [W, 3H] -> [128, GC, W]  (gate g3 = cchunk*128 + p)
            gw = np.ascontiguousarray(g.T.reshape(GC, 128, W).transpose(1, 0, 2))
            gxw[:, j * GC * W:(j + 1) * GC * W] = gw.reshape(128, GC * W)

        qoff = c * cfg.QB
        dm = np.zeros((cfg.QB, T), f32)
        for i in range(cfg.QB):
            dm[i, qoff + i] = NEG_BIG

        in_maps.append(dict(
            xT=xT.astype(bf16), wihT=wihT.astype(bf16), whhT=whhT,
            gxbias=gxbias, bhhn=bhhn,
            gxwarm=gxw.astype(bf16),
            diagmask=dm, wemitT=wemitT, bemit=bemit,
            ttrans_b=ttrans_b, tstart_b=tstart_b, tend_b=tend_b,
            lab16=tplane(lab), laba16=tplane(laba), labb16=tplane(labb),
            labends=np.tile(np.array([[lab[0], lab[T - 1]]], f32), (128, 1)),
        ))
    return in_maps


# ----------------------------------------------------------------------------
# Kernel builder
# ----------------------------------------------------------------------------

def build(nc: bacc.Bacc, tc: tile.TileContext, cfg: Cfg):
    T, E, H, U = cfg.T, cfg.E, cfg.H, cfg.U
    HC, GC, G3, QB, QTN, FW = cfg.HC, cfg.GC, cfg.G3, cfg.QB, cfg.QTN, cfg.FW
    CH, SW, W, Tsc, NSEG = cfg.CH, cfg.SW, cfg.W, cfg.Tsc, cfg.NSEG
    Tsc1 = Tsc + 1
    NK = E // 128
    NC = cfg.NC
    _ORDER = ["none", "gx", "scan", "ag", "attnq", "attnk", "attn",
              "emit", "ag3", None]
    lim = _ORDER.index(cfg.stop_after)

    def din(name, shape, dt=FP32):
        return nc.dram_tensor(name, list(shape), dt, kind="ExternalInput")

    xT_d = din("xT", (E, CH * SW), BF16)
    wihT_d = din("wihT", (E, G3), BF16)
    whhT_d = din("whhT", (H, G3), BF16)
    gxbias_d = din("gxbias", (128, GC))
    bhhn_d = din("bhhn", (128, HC))
    gxwarm_d = din("gxwarm", (128, CH * GC * W), BF16)
    diag_d = din("diagmask", (QB, T))
    wemitT_d = din("wemitT", (6 * H, 2), BF16)
    bemit_d = din("bemit", (128, 2))
    ttrans_d = din("ttrans_b", (128, 4))
    tstart_d = din("tstart_b", (128, 2))
    tend_d = din("tend_b", (128, 2))
    lab_d = din("lab16", (128, FW))
    laba_d = din("laba16", (128, FW))
    labb_d = din("labb16", (128, FW))
    labends_d = din("labends", (128, 2))

    out_d = nc.dram_tensor("out_scalar", [1, 1], FP32, kind="ExternalOutput")
    if cfg.debug_outs:
        dbg_emit_d = nc.dram_tensor("dbg_emit", [T, 2], FP32, kind="ExternalOutput")
        dbg_ys_d = nc.dram_tensor("dbg_ys", [CH * 2 * HC * 128, Tsc1], FP32,
                                  kind="ExternalOutput")

    pid = nc.partition_id()
    sel = pid % 2              # 1 on backward-direction cores

    # static map: true segment sg of direction dd -> (ag block, col base)
    # (host assigns bwd chains reversed segments, so this is dir-independent)
    def seg_src(dd, sg):
        return dd + 4 * (sg // CH), (sg % CH) * SW

    # ---- persistent small SBUF ----
    pers = tc.alloc_tile_pool(name="pers", bufs=1)
    whh_sb = pers.tile([128, HC * G3], BF16, tag="whh")
    gxbias_sb = pers.tile([128, GC], FP32, tag="gxbias")
    bhhn_sb = pers.tile([128, HC], FP32, tag="bhhn")
    ident = pers.tile([128, 128], FP32, tag="ident")
    make_identity(nc, ident[:])
    ident_bf = pers.tile([128, 128], BF16, tag="identbf")
    nc.vector.tensor_copy(ident_bf[:], ident[:])
    ttrans_sb = pers.tile([128, 4], FP32, tag="ttr")
    tstart_sb = pers.tile([128, 2], FP32, tag="tst")
    tend_sb = pers.tile([128, 2], FP32, tag="ten")
    lab_sb = pers.tile([128, FW], FP32, tag="lab")
    laba_sb = pers.tile([128, FW], FP32, tag="laba")
    labb_sb = pers.tile([128, FW], FP32, tag="labb")
    bemit_sb = pers.tile([128, 2], FP32, tag="bemit")
    wemit_sb = pers.tile([128, (6 * H // 128) * 2], BF16, tag="wemit")
    labends_sb = pers.tile([128, 2], FP32, tag="labends")

    nc.sync.dma_start(whh_sb[:], whhT_d.ap().rearrange("(k p) g -> p k g", p=128))
    nc.sync.dma_start(gxbias_sb[:], gxbias_d[:, :])
    nc.sync.dma_start(bhhn_sb[:], bhhn_d[:, :])
    nc.sync.dma_start(ttrans_sb[:], ttrans_d[:, :])
    nc.sync.dma_start(tstart_sb[:], tstart_d[:, :])
    nc.sync.dma_start(tend_sb[:], tend_d[:, :])
    nc.sync.dma_start(lab_sb[:], lab_d[:, :])
    nc.sync.dma_start(laba_sb[:], laba_d[:, :])
    nc.sync.dma_start(labb_sb[:], labb_d[:, :])
    nc.sync.dma_start(bemit_sb[:], bemit_d[:, :])
    nc.sync.dma_start(labends_sb[:], labends_d[:, :])
    nc.sync.dma_start(wemit_sb[:], wemitT_d.ap().rearrange("(k p) c -> p k c", p=128))

    # ---- DRAM pools for collectives ----
    dram = tc.alloc_tile_pool(name="dram", bufs=1, space="DRAM")
    b1_in = dram.tile([HC * 128, CH * SW], BF16, tag="b1i")
    ag1 = dram.tile([NC * HC * 128, CH * SW], BF16, tag="ag1", addr_space="Shared")
    b3_in = dram.tile([QB, 2], FP32, tag="b3i")
    ag3 = dram.tile([NC * QB, 2], FP32, tag="ag3", addr_space="Shared")

    ys_pool = tc.alloc_tile_pool(name="ysp", bufs=1)
    ys = ys_pool.tile([128, CH * 2 * HC * Tsc1], FP32, tag="ys")
    ysv = ys[:].rearrange("p (a t) -> p a t", a=CH * 2 * HC)

    # ============================ phase 1 + 2 =================================
    with tc.tile_pool(name="gxp", bufs=1) as gxp:
        gx_sb = gxp.tile([128, CH * GC * Tsc], BF16, tag="gx")
        gxv = gx_sb[:].rearrange("p (c t) -> p c t", c=CH * GC)

        # warmup gx columns from host
        nc.sync.dma_start(gxv[:, :, 0:W],
                          gxwarm_d.ap().rearrange("p (c w) -> p c w", c=CH * GC))

        if lim >= 1:
            with tc.tile_pool(name="ph1", bufs=1) as ph1, \
                 tc.tile_pool(name="ph1ps", bufs=2, space="PSUM") as ph1ps:
                xT_sb = ph1.tile([128, NK * CH * SW], BF16, tag="xT")
                wih_sb = ph1.tile([128, NK * G3], BF16, tag="wih")
                nc.sync.dma_start(
                    xT_sb[:], xT_d.ap().rearrange("(k p) t -> p k t", p=128))
                nc.sync.dma_start(
                    wih_sb[:], wihT_d.ap().rearrange("(k p) g -> p k g", p=128))

                CHK = min(SW, 512)
                for ch in range(CH):
                    for c in range(GC):
                        for n in range(SW // CHK):
                            ps = ph1ps.tile([128, CHK], FP32, tag="gxps")
                            for k in range(NK):
                                nc.tensor.matmul(
                                    ps[:, :],
                                    wih_sb[:, k * G3 + c * 128:
                                           k * G3 + (c + 1) * 128],
                                    xT_sb[:, k * CH * SW + ch * SW + n * CHK:
                                          k * CH * SW + ch * SW + (n + 1) * CHK],
                                    start=(k == 0), stop=(k == NK - 1))
                            nc.vector.tensor_scalar_add(
                                gxv[:, ch * GC + c,
                                    W + n * CHK:W + (n + 1) * CHK],
                                ps[:, :], gxbias_sb[:, c:c + 1])

        # ---- GRU scan: CH interleaved chains ----
        if lim >= 2:
            for ch in range(CH):
                nc.vector.memset(ysv[:, ch * 2 * HC:(ch + 1) * 2 * HC, 0:1], 0.0)
            skip_bhhn = cfg.skip_bhhn
            with tc.tile_pool(name="scan", bufs=3) as scp, \
                 tc.tile_pool(name="scanps0", bufs=2, space="PSUM") as pspl0, \
                 tc.tile_pool(name="scanps1", bufs=2, space="PSUM") as pspl1, \
                 tc.tile_pool(name="scanps2", bufs=2, space="PSUM") as pspl2, \
                 tc.tile_pool(name="scanps3", bufs=2, space="PSUM") as pspl3:
                pspl = [pspl0, pspl1, pspl2, pspl3][:CH]
                with tc.For_i(0, Tsc, U, staggered_reset=True,
                              hint_engines=(mybir.EngineType.PE,
                                            mybir.EngineType.DVE)) as iv:
                    hbf_prev = [None] * CH
                    for u in range(U):
                        j = iv + u
                        A0 = [ch * 2 * HC for ch in range(CH)]
                        G0 = [ch * GC for ch in range(CH)]
                        hbf = []
                        for ch in range(CH):
                            if u > 0:
                                hbf.append(hbf_prev[ch])
                            else:
                                h = scp.tile([128, HC], BF16, tag=f"hbf{ch}")
                                nc.vector.tensor_copy(
                                    h[:], ysv[:, A0[ch]:A0[ch] + HC, ds(j, 1)])
                                hbf.append(h)
                        pAB = [pspl[ch].tile([128, 12], FP32, tag=f"pAB{ch}",
                                             name=f"pAB{ch}")
                               for ch in range(CH)]
                        pA = [t[:, 0:8] for t in pAB]
                        pB = [t[:, 8:12] for t in pAB]
                        # r-gate matmuls, both chains
                        for ch in range(CH):
                            for c in range(0, 4):
                                for k in range(HC):
                                    nc.tensor.matmul(
                                        pA[ch][:, c:c + 1],
                                        whh_sb[:, k * G3 + c * 128:
                                               k * G3 + (c + 1) * 128],
                                        hbf[ch][:, k:k + 1], start=(k == 0),
                                        stop=False)
                                nc.tensor.matmul(
                                    pA[ch][:, c:c + 1], ident_bf[:],
                                    gxv[:, G0[ch] + c, ds(j, 1)],
                                    start=False, stop=True)
                        srz = [scp.tile([128, 8], FP32, tag=f"srz{ch}",
                                        name=f"srz{ch}") for ch in range(CH)]
                        sr = [t[:, 0:4] for t in srz]
                        sz = [t[:, 4:8] for t in srz]
                        # n- and z-gate matmuls, both chains
                        for ch in range(CH):
                            for c in range(8, 12):
                                for k in range(HC):
                                    nc.tensor.matmul(
                                        pB[ch][:, c - 8:c - 7],
                                        whh_sb[:, k * G3 + c * 128:
                                               k * G3 + (c + 1) * 128],
                                        hbf[ch][:, k:k + 1], start=(k == 0),
                                        stop=(k == HC - 1))
                            for c in range(4, 8):
                                for k in range(HC):
                                    nc.tensor.matmul(
                                        pA[ch][:, c:c + 1],
                                        whh_sb[:, k * G3 + c * 128:
                                               k * G3 + (c + 1) * 128],
                                        hbf[ch][:, k:k + 1], start=(k == 0),
                                        stop=False)
                                nc.tensor.matmul(
                                    pA[ch][:, c:c + 1], ident_bf[:],
                                    gxv[:, G0[ch] + c, ds(j, 1)],
                                    start=False, stop=True)
                        for ch in range(CH):
                            nc.scalar.activation(srz[ch][:], pA[ch][:, 0:8],
                                                 AF.Sigmoid)
                        tn2 = [scp.tile([128, HC], FP32, tag=f"tn2{ch}", name=f"tn2{ch}")
                               for ch in range(CH)]
                        tn3 = [scp.tile([128, HC], FP32, tag=f"tn3{ch}", name=f"tn3{ch}")
                               for ch in range(CH)]
                        for ch in range(CH):
                            if skip_bhhn:
                                nsrc = pB[ch][:, :]
                            else:
                                tn1 = scp.tile([128, HC], FP32, tag=f"tn1{ch}")
                                nc.vector.tensor_tensor(tn1[:], pB[ch][:, :],
                                                        bhhn_sb[:], ALU.add)
                                nsrc = tn1[:]
                            nc.vector.tensor_tensor(tn2[ch][:], nsrc, sr[ch],
                                                    ALU.mult)
                            nc.vector.tensor_tensor(
                                tn3[ch][:], tn2[ch][:],
                                gxv[:, G0[ch] + 8:G0[ch] + 12, ds(j, 1)], ALU.add)
                        nn = [scp.tile([128, HC], FP32, tag=f"nn{ch}", name=f"nn{ch}")
                              for ch in range(CH)]
                        for ch in range(CH):
                            nc.scalar.activation(nn[ch][:], tn3[ch][:], AF.Tanh)
                        t1 = [scp.tile([128, HC], FP32, tag=f"t1{ch}", name=f"t1{ch}")
                              for ch in range(CH)]
                        for ch in range(CH):
                            nc.gpsimd.tensor_tensor(
                                t1[ch][:], sz[ch],
                                ysv[:, A0[ch]:A0[ch] + HC, ds(j, 1)], ALU.mult)
                        for ch in range(CH):
                            t2 = scp.tile([128, HC], FP32, tag=f"t2{ch}")
                            nc.vector.scalar_tensor_tensor(
                                t2[:], sz[ch], 1.0, nn[ch][:],
                                ALU.subtract, ALU.mult)
                            hbf2 = scp.tile([128, HC], BF16, tag=f"hbf2{ch}")
                            nc.vector.tensor_tensor(hbf2[:], t1[ch][:], t2[:],
                                                    ALU.subtract)
                            nc.gpsimd.tensor_tensor(
                                ysv[:, A0[ch]:A0[ch] + HC, ds(j + 1, 1)],
                                t1[ch][:], t2[:], ALU.subtract)
                            nc.gpsimd.tensor_copy(
                                ysv[:, A0[ch] + HC:A0[ch] + 2 * HC,
                                    ds((Tsc - u) - iv, 1)],
                                ysv[:, A0[ch]:A0[ch] + HC, ds(j + 1, 1)])
                            hbf_prev[ch] = hbf2

    if cfg.debug_outs:
        nc.sync.dma_start(dbg_ys_d.ap().rearrange("(a p) t -> p a t", p=128),
                          ysv[:, :, :])

    # =================== phase 3: natural transposes + AGs ===================
    if lim >= 3:
        natp = tc.alloc_tile_pool(name="natp", bufs=1)
        conv = natp.tile([128, CH * HC * SW], BF16, tag="conv")
        for ch in range(CH):
            a0 = ch * 2 * HC
            nc.vector.tensor_copy(
                conv[:].rearrange("p (c k t) -> p c k t", c=CH, k=HC)[:, ch, :, :],
                ysv[:, ds(a0 + sel * HC, HC), ds(1 + W - W * sel, SW)])
        for ch in range(CH):
            nc.gpsimd.dma_start(
                b1_in[:, ch * SW:(ch + 1) * SW]
                .rearrange("(k p) t -> p k t", p=128),
                conv[:].rearrange("p (c k t) -> p c k t", c=CH, k=HC)[:, ch, :, :])
        nc.gpsimd.collective_compute(
            "AllGather", ALU.bypass, ins=[b1_in.opt()], outs=[ag1.opt()],
            replica_groups=[list(range(NC))])

        natp.release()
    ys_pool.release()

    # ===================== phase 4: attention (q-sharded) =====================
    att = tc.alloc_tile_pool(name="att", bufs=1)
    qt_sb = att.tile([128, 8 * QB], BF16, tag="qt")
    qt_cand = att.tile([128, 2 * 8 * QB], BF16, tag="qtc")
    diag_sb = att.tile([128, QTN * T], FP32, tag="diag")
    featsT = att.tile([128, 24 * QB], BF16, tag="featsT")
    pt_sb = att.tile([128, (T // 128) * QB], BF16, tag="ptq")
    natfull = att.tile([128, (T // 128) * 1024], BF16, tag="natfull")
    kfull = att.tile([128, 8 * T], BF16, tag="kfull")
    emit_sb = att.tile([128, QTN * 2], FP32, tag="emit")

    if lim >= 4:
        # queries: rows [pid*QB, pid*QB+QB) of tgt features, runtime-stitched
        ratio = SW // QB                       # power of 2
        sh = ratio.bit_length() - 1
        sgq = pid >> sh                        # true segment of this q range
        oq = (pid & (ratio - 1)) * QB          # offset within segment
        cb = nc.s_assert_within((sgq & (CH - 1)) * SW + oq, 0, CH * SW - QB,
                                skip_runtime_assert=True)
        # runtime rows on a Shared tile crash the DMA engine: load both core
        # halves' candidate blocks (static rows, runtime col) and select after
        for hh in range(2):
            nc.gpsimd.dma_start(
                qt_cand[:, hh * 8 * QB:(hh + 1) * 8 * QB]
                .rearrange("p (k q) -> p k q", k=8),
                ag1[(2 + 4 * hh) * 512:(4 + 4 * hh) * 512, ds(cb, QB)]
                .rearrange("(k p) t -> p k t", p=128))
        sgh = sgq >> (CH.bit_length() - 1)     # which core half owns it
        qoff = nc.s_assert_within(sgh * (8 * QB), 0, 8 * QB,
                                  skip_runtime_assert=True)
        nc.vector.tensor_copy(qt_sb[:], qt_cand[:, ds(qoff, 8 * QB)])
        nc.vector.tensor_copy(featsT[:, 0:8 * QB], qt_sb[:])
        nc.vector.tensor_scalar_mul(qt_sb[:], qt_sb[:], float(np.sqrt(2.0 * H)))
        nc.sync.dma_start(diag_sb[:].rearrange("p (q t) -> p q t", q=QTN),
                          diag_d.ap().rearrange("(q p) t -> p q t", p=128))

    with tc.tile_pool(name="psS", bufs=1, space="PSUM") as psS, \
         tc.tile_pool(name="psT", bufs=2, space="PSUM") as psT, \
         tc.tile_pool(name="psC", bufs=2, space="PSUM") as psC, \
         tc.tile_pool(name="Pp", bufs=2) as Pp, \
         tc.tile_pool(name="attsm", bufs=4) as attsm:
        if lim >= 5:
            ST = T // 128
            for at in range(2):
                if at == 1 and lim < 6:
                    continue
                kv0 = 0 if at == 0 else 2
                # K cache: [feat-chunk kt][true t] for this attention's 2 dirs
                for dpart in range(2):
                    for sg in range(NSEG):
                        blk, cb = seg_src(kv0 + dpart, sg)
                        nc.sync.dma_start(
                            kfull[:, :].rearrange("p (a t) -> p a t", a=8)
                            [:, dpart * NK:(dpart + 1) * NK, sg * SW:(sg + 1) * SW],
                            ag1[blk * 512:(blk + 1) * 512, cb:cb + SW]
                            .rearrange("(k p) t -> p k t", p=128))
                # V natural tiles: transpose the K cache on-core
                for st in range(ST):
                    for m in range(8):
                        kt = (m // NK) * NK + (m % NK)
                        ntp = psT.tile([128, 128], BF16, tag="ptp")
                        nc.tensor.transpose(
                            ntp[:], kfull[:, kt * T + st * 128:
                                          kt * T + (st + 1) * 128], ident_bf[:])
                        nc.vector.tensor_copy(
                            natfull[:, st * 1024 + m * 128:
                                    st * 1024 + (m + 1) * 128], ntp[:])
                if lim < 6:
                    continue
                for qi in range(QTN):
                    pS = [psS.tile([128, T // 2], FP32, tag=f"pS{sh}",
                                   name=f"pS{sh}") for sh in range(2)]
                    for kt in range(8):
                        for nch in range(T // 512):
                            sh = nch // (T // 1024)
                            off = (nch % (T // 1024)) * 512
                            nc.tensor.matmul(
                                pS[sh][:, off:off + 512],
                                qt_sb[:, kt * QB + qi * 128:
                                      kt * QB + (qi + 1) * 128],
                                kfull[:, kt * T + nch * 512:
                                      kt * T + (nch + 1) * 512],
                                start=(kt == 0), stop=(kt == 7))
                    if at == 1:
                        for sh in range(2):
                            nc.vector.tensor_tensor(
                                pS[sh][:, :], pS[sh][:, :],
                                diag_sb[:, qi * T + sh * (T // 2):
                                        qi * T + (sh + 1) * (T // 2)],
                                ALU.add)
                    mx = [attsm.tile([128, 1], FP32, tag=f"mx{sh}",
                                     name=f"mx{sh}") for sh in range(2)]
                    for sh in range(2):
                        nc.vector.reduce_max(mx[sh][:], pS[sh][:, :], AX.X)
                    negm = attsm.tile([128, 1], FP32, tag="negm")
                    nc.vector.tensor_tensor(negm[:], mx[0][:], mx[1][:], ALU.max)
                    nc.vector.tensor_scalar_mul(negm[:], negm[:], -1.0)
                    Pt = Pp.tile([128, T], FP32, tag="P")
                    Ptb = Pp.tile([128, T], BF16, tag="Pb")
                    sm = [attsm.tile([128, 1], FP32, tag=f"sm{sh}",
                                     name=f"sm{sh}") for sh in range(2)]
                    for sh in range(2):
                        nc.scalar.activation(
                            Pt[:, sh * (T // 2):(sh + 1) * (T // 2)],
                            pS[sh][:, :], AF.Exp, bias=negm[:],
                            accum_out=sm[sh][:])
                    smc = attsm.tile([128, 1], FP32, tag="smc")
                    nc.vector.tensor_tensor(smc[:], sm[0][:], sm[1][:], ALU.add)
                    rinv = attsm.tile([128, 1], FP32, tag="rinv")
                    nc.vector.reciprocal(rinv[:], smc[:])
                    nc.vector.tensor_scalar_mul(Ptb[:, :], Pt[:, :], rinv[:])
                    for st in range(ST):
                        ptp = psT.tile([128, 128], BF16, tag="ptp")
                        nc.tensor.transpose(ptp[:], Ptb[:, st * 128:(st + 1) * 128],
                                            ident_bf[:])
                        nc.vector.tensor_copy(
                            pt_sb[:, st * QB + qi * 128:
                                  st * QB + (qi + 1) * 128],
                            ptp[:])
                for m in range(8):
                    pc = psC.tile([128, QB], FP32, tag="pc")
                    for st in range(ST):
                        nc.tensor.matmul(
                            pc[:],
                            natfull[:, st * 1024 + m * 128:
                                    st * 1024 + (m + 1) * 128],
                            pt_sb[:, st * QB:(st + 1) * QB],
                            start=(st == 0), stop=(st == ST - 1))
                    nc.vector.tensor_copy(
                        featsT[:, (8 + at * 8 + m) * QB:(9 + at * 8 + m) * QB],
                        pc[:])

        if lim >= 7:
            for qi in range(QTN):
                pe = psC.tile([128, 2], FP32, tag="pc")
                for kt in range(24):
                    nc.tensor.matmul(
                        pe[:, :],
                        featsT[:, kt * QB + qi * 128: kt * QB + (qi + 1) * 128],
                        wemit_sb[:, kt * 2:(kt + 1) * 2],
                        start=(kt == 0), stop=(kt == 23))
                nc.vector.tensor_tensor(emit_sb[:, qi * 2:(qi + 1) * 2], pe[:, :],
                                        bemit_sb[:], ALU.add)

    if lim >= 8:
        nc.gpsimd.dma_start(b3_in[:].rearrange("(q p) c -> p q c", p=128),
                            emit_sb[:].rearrange("p (q c) -> p q c", q=QTN))
        nc.gpsimd.collective_compute(
            "AllGather", ALU.bypass, ins=[b3_in.opt()], outs=[ag3.opt()],
            replica_groups=[list(range(NC))])
    if cfg.debug_outs:
        nc.sync.dma_start(dbg_emit_d[:, :], ag3[0:T, :])

    # ========================= phase 5: CRF + gold ===========================
    if lim >= 9:
        crf = tc.alloc_tile_pool(name="crf", bufs=1)
        crfps = tc.alloc_tile_pool(name="crfps", bufs=2, space="PSUM")
        ep = [crf.tile([128, FW], FP32, tag=f"ep{i}", name=f"ep{i}")
              for i in range(2)]
        for i in range(2):
            nc.sync.dma_start(
                ep[i][:],
                ag3[0:T, :].rearrange("(p f) c -> p f c", p=128)[:, :, i:i + 1])

        pl = [[crf.tile([128, FW], FP32, tag=f"pl{i}{j}", name=f"pl{i}{j}")
               for j in range(2)] for i in range(2)]
        for i in range(2):
            for j in range(2):
                nc.vector.tensor_scalar_add(pl[i][j][:], ep[i][:],
                                            ttrans_sb[:, 2 * i + j: 2 * i + j + 1])
        for i in range(2):
            for j in range(2):
                nc.vector.tensor_tensor(pl[i][j][0:1, 0:1], ep[i][0:1, 0:1],
                                        tstart_sb[0:1, i:i + 1], ALU.add)

        cur = pl
        Wf = FW
        lvl = 0
        while Wf > 1:
            Wf //= 2
            nxt = [[crf.tile([128, Wf], FP32, tag=f"lv{lvl}_{i}{j}",
                             name=f"lv{lvl}_{i}{j}") for j in range(2)]
                   for i in range(2)]
            Aap = [[cur[k][j][:, 0:2 * Wf]
                    .rearrange("p (m two) -> p m two", two=2)[:, :, 0:1]
                    for j in range(2)] for k in range(2)]
            Bap = [[cur[i][k][:, 0:2 * Wf]
                    .rearrange("p (m two) -> p m two", two=2)[:, :, 1:2]
                    for k in range(2)] for i in range(2)]
            for i in range(2):
                for j in range(2):
                    X = crf.tile([128, Wf], FP32, tag=f"Xf{lvl}{i}{j}")
                    Y = crf.tile([128, Wf], FP32, tag=f"Yf{lvl}{i}{j}")
                    nc.vector.tensor_tensor(X[:], Bap[i][0], Aap[0][j], ALU.add)
                    nc.vector.tensor_tensor(Y[:], Bap[i][1], Aap[1][j], ALU.add)
                    M = crf.tile([128, Wf], FP32, tag=f"Mf{lvl}{i}{j}")
                    nc.vector.tensor_tensor(M[:], X[:], Y[:], ALU.max)
                    mn = crf.tile([128, Wf], FP32, tag=f"mnf{lvl}{i}{j}")
                    nc.vector.tensor_tensor(mn[:], X[:], Y[:], ALU.min)
                    dm = crf.tile([128, Wf], FP32, tag=f"dmf{lvl}{i}{j}")
                    nc.vector.tensor_tensor(dm[:], mn[:], M[:], ALU.subtract)
                    spe = crf.tile([128, Wf], FP32, tag=f"spef{lvl}{i}{j}")
                    nc.scalar.activation(spe[:], dm[:], AF.Exp)
                    sp = crf.tile([128, Wf], FP32, tag=f"spf{lvl}{i}{j}")
                    nc.scalar.activation(sp[:], spe[:], AF.Ln, bias=1.0)
                    nc.vector.tensor_tensor(nxt[i][j][:], M[:], sp[:], ALU.add)
            cur = nxt
            lvl += 1

        r128 = [[None, None], [None, None]]
        for i in range(2):
            for j in range(2):
                tps = crfps.tile([128, 128], FP32, tag="tps", name=f"tps{i}{j}")
                nc.tensor.transpose(tps[0:1, :], cur[i][j][:, 0:1], ident[:])
                rr = crf.tile([1, 128], FP32, tag=f"r128_{i}{j}",
                              name=f"r128_{i}{j}")
                nc.vector.tensor_copy(rr[:], tps[0:1, :])
                r128[i][j] = rr

        curp = r128
        curW = 128
        lvl = 0
        while curW > 1:
            curW //= 2
            nxtp = [[crf.tile([1, curW], FP32, tag=f"p4_{lvl}{i}{j}",
                              name=f"p4_{lvl}{i}{j}") for j in range(2)]
                    for i in range(2)]
            Aap = [[curp[k][j][0:1, 0:2 * curW]
                    .rearrange("p (m two) -> p m two", two=2)[:, :, 0:1]
                    for j in range(2)] for k in range(2)]
            Bap = [[curp[i][k][0:1, 0:2 * curW]
                    .rearrange("p (m two) -> p m two", two=2)[:, :, 1:2]
                    for k in range(2)] for i in range(2)]
            for i in range(2):
                for j in range(2):
                    X = crf.tile([1, curW], FP32, tag=f"X4{lvl}{i}{j}",
                                 name=f"X4{lvl}{i}{j}")
                    Y = crf.tile([1, curW], FP32, tag=f"Y4{lvl}{i}{j}",
                                 name=f"Y4{lvl}{i}{j}")
                    nc.vector.tensor_tensor(X[:], Bap[i][0], Aap[0][j], ALU.add)
                    nc.vector.tensor_tensor(Y[:], Bap[i][1], Aap[1][j], ALU.add)
                    M = crf.tile([1, curW], FP32, tag=f"M4{lvl}{i}{j}",
                                 name=f"M4{lvl}{i}{j}")
                    nc.vector.tensor_tensor(M[:], X[:], Y[:], ALU.max)
                    mn = crf.tile([1, curW], FP32, tag=f"mn4{lvl}{i}{j}",
                                  name=f"mn4{lvl}{i}{j}")
                    nc.vector.tensor_tensor(mn[:], X[:], Y[:], ALU.min)
                    dm = crf.tile([1, curW], FP32, tag=f"dm4{lvl}{i}{j}",
                                  name=f"dm4{lvl}{i}{j}")
                    nc.vector.tensor_tensor(dm[:], mn[:], M[:], ALU.subtract)
                    spe = crf.tile([1, curW], FP32, tag=f"spe4{lvl}{i}{j}",
                                   name=f"spe4{lvl}{i}{j}")
                    nc.scalar.activation(spe[:], dm[:], AF.Exp)
                    sp = crf.tile([1, curW], FP32, tag=f"sp4{lvl}{i}{j}",
                                  name=f"sp4{lvl}{i}{j}")
                    nc.scalar.activation(sp[:], spe[:], AF.Ln, bias=1.0)
                    nc.vector.tensor_tensor(nxtp[i][j][:], M[:], sp[:], ALU.add)
            curp = nxtp
            lvl += 1

        sc = crf.tile([1, 16], FP32, tag="scratch")

        def s_op(dst, a, b, op):
            nc.vector.tensor_tensor(dst, a, b, op)

        a0_ = sc[0:1, 0:1]; a1_ = sc[0:1, 1:2]
        s_op(a0_, curp[0][0][0:1, 0:1], tend_sb[0:1, 0:1], ALU.add)
        s_op(a1_, curp[1][0][0:1, 0:1], tend_sb[0:1, 1:2], ALU.add)
        M_ = sc[0:1, 2:3]; mn_ = sc[0:1, 3:4]; dm_ = sc[0:1, 4:5]; sp_ = sc[0:1, 5:6]
        s_op(M_, a0_, a1_, ALU.max)
        s_op(mn_, a0_, a1_, ALU.min)
        s_op(dm_, mn_, M_, ALU.subtract)
        spe_ = sc[0:1, 13:14]
        nc.scalar.activation(spe_, dm_, AF.Exp)
        nc.scalar.activation(sp_, spe_, AF.Ln, bias=1.0)
        logz = sc[0:1, 6:7]
        s_op(logz, M_, sp_, ALU.add)

        # ---- gold ----
        gsc = crf.tile([128, FW], FP32, tag="goldscratch")
        parts = crf.tile([128, 8], FP32, tag="parts")
        nc.vector.memset(parts[:], 0.0)
        ge = crf.tile([128, FW], FP32, tag="ge")
        nc.vector.tensor_tensor(ge[:], ep[1][:], ep[0][:], ALU.subtract)
        nc.vector.reduce_sum(parts[:, 0:1], ep[0][:], AX.X)
        nc.vector.scalar_tensor_tensor(gsc[:], ge[:], 1.0, lab_sb[:], ALU.mult,
                                       ALU.mult, accum_out=parts[:, 1:2])
        nc.vector.reduce_sum(parts[:, 2:3], laba_sb[:], AX.X)
        nc.vector.reduce_sum(parts[:, 3:4], labb_sb[:], AX.X)
        nc.vector.scalar_tensor_tensor(gsc[:], laba_sb[:], 1.0, labb_sb[:],
                                       ALU.mult, ALU.mult,
                                       accum_out=parts[:, 4:5])
        sums_ps = crfps.tile([1, 8], FP32, tag="sumsps")
        ones = crf.tile([128, 1], FP32, tag="ones")
        nc.vector.memset(ones[:], 1.0)
        nc.tensor.matmul(sums_ps[:], ones[:], parts[:], start=True, stop=True)
        sums = crf.tile([1, 8], FP32, tag="sums")
        nc.vector.tensor_copy(sums[:], sums_ps[:])

        l0 = labends_sb[0:1, 0:1]
        llast = labends_sb[0:1, 1:2]
        dts = sc[0:1, 7:8]; m1 = sc[0:1, 8:9]; tstart_t = sc[0:1, 9:10]
        s_op(dts, tstart_sb[0:1, 1:2], tstart_sb[0:1, 0:1], ALU.subtract)
        s_op(m1, l0, dts, ALU.mult)
        s_op(tstart_t, m1, tstart_sb[0:1, 0:1], ALU.add)
        dte = sc[0:1, 10:11]; m2 = sc[0:1, 11:12]; tend_t = sc[0:1, 12:13]
        s_op(dte, tend_sb[0:1, 1:2], tend_sb[0:1, 0:1], ALU.subtract)
        s_op(m2, llast, dte, ALU.mult)
        s_op(tend_t, m2, tend_sb[0:1, 0:1], ALU.add)

        sc2 = crf.tile([1, 16], FP32, tag="scratch2")
        dA = sc2[0:1, 0:1]; dB = sc2[0:1, 1:2]; dAB = sc2[0:1, 2:3]
        e1 = sc2[0:1, 3:4]
        s_op(dA, ttrans_sb[0:1, 2:3], ttrans_sb[0:1, 0:1], ALU.subtract)
        s_op(dB, ttrans_sb[0:1, 1:2], ttrans_sb[0:1, 0:1], ALU.subtract)
        s_op(e1, ttrans_sb[0:1, 3:4], ttrans_sb[0:1, 2:3], ALU.subtract)
        s_op(dAB, e1, dB, ALU.subtract)
        t00s = sc2[0:1, 4:5]
        nc.scalar.mul(t00s, ttrans_sb[0:1, 0:1], float(T - 1))
        tA = sc2[0:1, 5:6]; tB = sc2[0:1, 6:7]; tAB = sc2[0:1, 7:8]
        s_op(tA, sums[0:1, 2:3], dA, ALU.mult)
        s_op(tB, sums[0:1, 3:4], dB, ALU.mult)
        s_op(tAB, sums[0:1, 4:5], dAB, ALU.mult)
        acc1 = sc2[0:1, 8:9]; acc2 = sc2[0:1, 9:10]; acc3 = sc2[0:1, 10:11]
        s_op(acc1, t00s, tA, ALU.add)
        s_op(acc2, acc1, tB, ALU.add)
        s_op(acc3, acc2, tAB, ALU.add)
        g1 = sc2[0:1, 11:12]; g2 = sc2[0:1, 12:13]; g3 = sc2[0:1, 13:14]
        g4 = sc2[0:1, 14:15]
        s_op(g1, tstart_t, sums[0:1, 0:1], ALU.add)
        s_op(g2, g1, sums[0:1, 1:2], ALU.add)
        s_op(g3, g2, acc3, ALU.add)
        s_op(g4, g3, tend_t, ALU.add)
        res = sc2[0:1, 15:16]
        s_op(res, g4, logz, ALU.subtract)
        nc.sync.dma_start(out_d[0:1, 0:1], res)
        crfps.release()
        crf.release()
    else:
        nc.sync.dma_start(out_d[0:1, 0:1], tstart_sb[0:1, 0:1])
    att.release()
    dram.release()
    pers.release()


def build_program(cfg: Cfg):
    nc = bacc.Bacc("TRN2", target_bir_lowering=False, debug=False,
                   num_devices=cfg.NC)
    with tile.TileContext(nc) as tc:
        build(nc, tc, cfg)
    nc.compile()
    return nc


# ============================================================================
# Harness entry point
# ============================================================================

_CACHE = {}


def _get_program(cfg_key, cfg):
    if cfg_key not in _CACHE:
        _CACHE[cfg_key] = build_program(cfg)
    return _CACHE[cfg_key]


def kernel(**inputs):
    """Full-input kernel: shards across 8 NeuronCores internally."""
    from concourse import bass_utils

    bhh_zero = all(
        not np.any(np.asarray(inputs[f"{enc}_bhh_{dr}"])[2 * 512:])
        for enc in ("src", "tgt") for dr in ("f", "b"))
    cfg = Cfg(T=2048, U=32, W=32, skip_bhhn=bhh_zero)
    nc = _get_program(("main", bhh_zero), cfg)
    in_maps = prep_in_maps(inputs, cfg)
    res = bass_utils.run_bass_kernel_spmd(
        nc, in_maps, core_ids=list(range(cfg.NC)))
    out = np.asarray(res.results[0]["out_scalar"], dtype=np.float32)
    return out.reshape(())
